# revision 1
# baseline (speedup 1.0000x reference)
"""Multi-head causal attention on 8 Trainium2 NeuronCores.

Sharding: core c handles batch b = c // 2 and head-group g = c % 2
(8 of 16 heads, i.e. 512 of 1024 projection columns).  QKV projections,
attention and the output projection partial run per-core; the two cores
of a batch pair-ReduceScatter their partial outputs.

Everything on-device is computed in a transposed layout (seq on the
free dim) so no PE transposes are needed anywhere:
  xT [D, L] (host-pre-transposed) -> qT/kT [512, L] -> S^T [keys, q]
  -> P^T = exp(S^T) (bf16) -> attn^T = (v|ones)^T @ P^T (Z row free)
  -> out^T = Wo^T @ attn_norm^T.  Host un-transposes the result.
"""

import sys, types

sys.path.insert(0, "/opt/trn_rl_repo")

# antenv.axon_hooks is missing in this image; inject it so trace=True can
# reach the NTFF profiling hook (used by test.py, off by default).
if "antenv.axon_hooks" not in sys.modules:
    _hook_mod = types.ModuleType("antenv.axon_hooks")
    _hook_mod._hook = None
    def _set_hook(h):
        _hook_mod._hook = h
    def _get_hook():
        return _hook_mod._hook
    _hook_mod.set_axon_ntff_profile_hook = _set_hook
    _hook_mod.get_axon_ntff_profile_hook = _get_hook
    sys.modules["antenv.axon_hooks"] = _hook_mod
    try:
        import antenv
        antenv.axon_hooks = _hook_mod
        from trn_agent_boot.trn_boot import _ntff_profile_via_ctypes
        _set_hook(_ntff_profile_via_ctypes("/opt/axon/libaxon_pjrt.so"))
    except Exception:
        pass

import numpy as np
import ml_dtypes
import concourse.bass as bass
import concourse.mybir as mybir
import concourse.tile as tile
from concourse import bacc
from concourse.bass_utils import run_bass_kernel_spmd

B, L, D, H = 4, 2048, 1024, 16
DH = 64
N_CORES = 8
NH = 8          # heads per core
HC = NH * DH    # 512 projection cols per core
QC = 512        # q-chunk
KT = 128        # k-tile
P = 128

F32 = mybir.dt.float32
F32R = mybir.dt.float32r
BF16 = mybir.dt.bfloat16

TRACE = False
LAST_EXEC_NS = None
_NC = None


def build_nc(seq_len=L):
    Ls = seq_len
    NQC = Ls // QC
    NKT = Ls // KT
    NDS = D // P       # 8 contraction tiles for projections
    nc = bacc.Bacc()

    xT = nc.declare_dram_parameter("xT", [D, Ls], F32R, isOutput=False)
    wq = nc.declare_dram_parameter("wq", [D, HC], F32R, isOutput=False)
    wk = nc.declare_dram_parameter("wk", [D, HC], F32R, isOutput=False)
    wv = nc.declare_dram_parameter("wv", [D, HC], F32R, isOutput=False)
    wo = nc.declare_dram_parameter("wo", [HC, D], BF16, isOutput=False)
    bq = nc.declare_dram_parameter("bq", [P, HC // P], F32, isOutput=False)
    bk = nc.declare_dram_parameter("bk", [P, HC // P], F32, isOutput=False)
    bv = nc.declare_dram_parameter("bv", [P, HC], F32, isOutput=False)
    bo = nc.declare_dram_parameter("bo", [P, D // P], F32, isOutput=False)
    m01 = nc.declare_dram_parameter("m01", [P, 4 * QC], BF16, isOutput=False)
    outTh = nc.declare_dram_parameter("outTh", [D // 2, Ls], F32, isOutput=True)

    partT = nc.dram_tensor("partT", [D, Ls], F32)
    rs_out = nc.dram_tensor("rs_out", [D // 2, Ls], F32)

    scale = 1.0 / np.sqrt(np.float32(DH))

    from contextlib import ExitStack
    with nc.allow_low_precision(reason="f32r matmul inputs; bf16 P/V by design"), \
         tile.TileContext(nc) as tc, ExitStack() as ctx:
        consts = ctx.enter_context(tc.tile_pool(name="consts", bufs=1))
        wpool = ctx.enter_context(tc.tile_pool(name="wpool", bufs=1))
        kvres = ctx.enter_context(tc.tile_pool(name="kvres", bufs=1))
        xtp = ctx.enter_context(tc.tile_pool(name="xtp", bufs=8))
        qtp = ctx.enter_context(tc.tile_pool(name="qtp", bufs=8))
        ptp = ctx.enter_context(tc.tile_pool(name="ptp", bufs=8))
        anp = ctx.enter_context(tc.tile_pool(name="anp", bufs=8))
        otp = ctx.enter_context(tc.tile_pool(name="otp", bufs=3))
        zrp = ctx.enter_context(tc.tile_pool(name="zrp", bufs=2))
        bzsb = ctx.enter_context(tc.tile_pool(name="bzsb", bufs=2))
        anodd = ctx.enter_context(tc.tile_pool(name="anodd", bufs=2))
        zdp = ctx.enter_context(tc.tile_pool(name="zdp", bufs=4, space="DRAM"))
        scratch = ctx.enter_context(tc.tile_pool(name="scratch", bufs=2, space="PSUM"))
        stp = ctx.enter_context(tc.tile_pool(name="stp", bufs=2, space="PSUM"))
        accp = ctx.enter_context(tc.tile_pool(name="accp", bufs=2, space="PSUM"))

        if True:
            # ---- constants ----
            bq_sb = consts.tile([P, HC // P], F32, tag="bq")
            bk_sb = consts.tile([P, HC // P], F32, tag="bk")
            bv_sb = consts.tile([P, HC], F32, tag="bv")
            bo_sb = consts.tile([P, D // P], F32, tag="bo")
            m01_sb = consts.tile([P, 4, QC], BF16, tag="m01")
            ones_sb = consts.tile([P, DH], F32, tag="ones")
            nc.sync.dma_start(out=bq_sb, in_=bq[:, :])
            nc.sync.dma_start(out=bk_sb, in_=bk[:, :])
            nc.sync.dma_start(out=bv_sb, in_=bv[:, :])
            nc.sync.dma_start(out=bo_sb, in_=bo[:, :])
            nc.sync.dma_start(out=m01_sb, in_=m01[:, :].rearrange("p (m q) -> p m q", m=4))
            nc.vector.memset(ones_sb, 1.0)

            # ---- weights resident ----
            wq_sb = [wpool.tile([P, HC], F32R, tag=f"wq{ds}", name=f"wq{ds}") for ds in range(NDS)]
            wk_sb = [wpool.tile([P, HC], F32R, tag=f"wk{ds}", name=f"wk{ds}") for ds in range(NDS)]
            wv_sb = [wpool.tile([P, HC], F32R, tag=f"wv{ds}", name=f"wv{ds}") for ds in range(NDS)]
            wo_sb = [wpool.tile([P, D], BF16, tag=f"wo{t}", name=f"wo{t}") for t in range(HC // P)]
            for ds in range(NDS):
                nc.sync.dma_start(out=wq_sb[ds], in_=wq[ds * P:(ds + 1) * P, :])
                nc.sync.dma_start(out=wk_sb[ds], in_=wk[ds * P:(ds + 1) * P, :])
                nc.sync.dma_start(out=wv_sb[ds], in_=wv[ds * P:(ds + 1) * P, :])
            for t in range(HC // P):
                nc.sync.dma_start(out=wo_sb[t], in_=wo[t * P:(t + 1) * P, :])

            # ---- resident kT and v ----
            kT_sb = [kvres.tile([P, Ls], F32R, tag=f"kT{t}", name=f"kT{t}") for t in range(HC // P)]
            # v: per key-tile [128, NH, 65] bf16; cols 0..63 = v, col 64 = ones
            # (the ones column makes the AV matmul emit softmax Z in row 64)
            v_sb = [kvres.tile([P, NH, 65], BF16, tag=f"v{kt}", name=f"v{kt}") for kt in range(NKT)]
            for kt in range(NKT):
                nc.vector.memset(v_sb[kt], 1.0)

            # ---- per-chunk: projections for chunk s, then attention and
            # output projection for q-chunk c=s (causal => only needs k/v
            # from chunks <= s) ----
            xT_t = {}
            qT_t = {}
            attn_by_chunk = {}
            for s in range(NQC):
                for ds in range(NDS):
                    xt = xtp.tile([P, QC], F32R, tag="xT")
                    nc.sync.dma_start(
                        out=xt, in_=xT[ds * P:(ds + 1) * P, s * QC:(s + 1) * QC])
                    xT_t[(ds, s)] = xt

                for t in range(HC // P):
                    # qT tile [128 outcol, QC seq]
                    pq = scratch.tile([P, QC], F32, tag="pacc")
                    for ds in range(NDS):
                        nc.tensor.matmul(
                            pq,
                            wq_sb[ds][:, t * P:(t + 1) * P],
                            xT_t[(ds, s)],
                            start=(ds == 0), stop=(ds == NDS - 1))
                    qt = qtp.tile([P, QC], F32R, tag="qT")
                    # scale q by 1/sqrt(dh) here; add bias then scale:
                    # (q+b)*s = func(in*s + b*s) with pre-scaled bias
                    nc.scalar.activation(
                        out=qt, in_=pq,
                        func=mybir.ActivationFunctionType.Identity,
                        bias=bq_sb[:, t:t + 1], scale=1.0)
                    qT_t[(t, s)] = qt

                    pk = scratch.tile([P, QC], F32, tag="pacc")
                    for ds in range(NDS):
                        nc.tensor.matmul(
                            pk,
                            wk_sb[ds][:, t * P:(t + 1) * P],
                            xT_t[(ds, s)],
                            start=(ds == 0), stop=(ds == NDS - 1))
                    nc.scalar.activation(
                        out=kT_sb[t][:, s * QC:(s + 1) * QC], in_=pk,
                        func=mybir.ActivationFunctionType.Identity,
                        bias=bk_sb[:, t:t + 1], scale=1.0)

                # v for the 4 key-tiles of this seq chunk
                for sub in range(QC // P):
                    kt = s * (QC // P) + sub
                    pv = scratch.tile([P, HC], F32, tag="pacc")
                    for ds in range(NDS):
                        nc.tensor.matmul(
                            pv,
                            xT_t[(ds, s)][:, sub * P:(sub + 1) * P],
                            wv_sb[ds],
                            start=(ds == 0), stop=(ds == NDS - 1))
                    nc.vector.tensor_add(
                        v_sb[kt][:, :, 0:64],
                        pv[:].rearrange("p (h d) -> p h d", h=NH),
                        bv_sb[:].rearrange("p (h d) -> p h d", h=NH))

                # ---- attention + output projection for q-chunk c = s ----
                c = s
                njt = min(4 * c + 4, NKT)     # causal: k-tiles 0..4c+3
                ngrp = (njt + 1) // 2
                attn_n = {}
                for t in range(HC // P):
                    an_t = anp.tile([P, QC], BF16, tag="an")
                    for par in range(2):
                        h = 2 * t + par
                        # S^T and P^T for all k-tile groups of this head
                        pts = []
                        for g in range(ngrp):
                            st = stp.tile([P, 2 * QC], F32, tag="st")
                            for half in range(2):
                                j = 2 * g + half
                                if j >= njt:
                                    continue
                                nc.tensor.matmul(
                                    st[:, half * QC:(half + 1) * QC],
                                    kT_sb[t][par * DH:(par + 1) * DH,
                                             j * KT:(j + 1) * KT],
                                    qT_t[(t, c)][par * DH:(par + 1) * DH, :],
                                    start=True, stop=True)
                            pt = ptp.tile([P, 2 * QC], BF16, tag="pt")
                            # exp(scale * s)
                            ncols = QC * (2 if 2 * g + 1 < njt else 1)
                            nc.scalar.activation(
                                out=pt[:, :ncols], in_=st[:, :ncols],
                                func=mybir.ActivationFunctionType.Exp,
                                scale=float(scale))
                            pts.append(pt)
                            for half in range(2):
                                j = 2 * g + half
                                if j >= njt or j < 4 * c:
                                    continue
                                m = j - 4 * c
                                nc.vector.tensor_mul(
                                    pt[:, half * QC:(half + 1) * QC],
                                    pt[:, half * QC:(half + 1) * QC],
                                    m01_sb[:, m, :])
                        # AV with fused ones column -> rows 0..63 attn, row 64 = Z
                        acc = accp.tile([P, QC], F32, tag="acc")
                        for j in range(njt):
                            nc.tensor.matmul(
                                acc[0:65, :],
                                v_sb[j][:, h, :],
                                pts[j // 2][:, (j % 2) * QC:(j % 2 + 1) * QC],
                                start=(j == 0), stop=(j == njt - 1))
                        # normalization: zrec = 1/Z, broadcast over 64 rows via PE
                        zrec = zrp.tile([P, QC], F32, tag="zrec")
                        nc.vector.reciprocal(
                            out=zrec[64:65, :], in_=acc[64:65, :])
                        bzs = bzsb.tile([P, QC], F32, tag="bzs")
                        zrow = zdp.tile([1, QC], F32, tag="zd", name="zrow")
                        nc.sync.dma_start(out=zrow, in_=zrec[64:65, :])
                        nc.sync.dma_start(out=bzs[0:DH, :].unsqueeze(1),
                                          in_=zrow.partition_broadcast(DH))
                        if par == 0:
                            nc.vector.tensor_mul(
                                an_t[0:DH, :], acc[0:DH, :], bzs[0:DH, :])
                        else:
                            an_o = anodd.tile([DH, QC], BF16, tag="anodd")
                            nc.vector.tensor_mul(
                                an_o, acc[0:DH, :], bzs[0:DH, :])
                            # shift to partitions 64..127 (DMA can cross lanes)
                            nc.sync.dma_start(out=an_t[DH:P, :], in_=an_o)
                    attn_n[t] = an_t

                attn_by_chunk[c] = attn_n
                # output projection, delayed one chunk so the (slow) softmax
                # normalization chain of chunk c overlaps proj matmuls of c+1
                for oc in ([c - 1] if c + 1 < NQC else [c - 1, c]):
                    if oc < 0:
                        continue
                    an_c = attn_by_chunk.pop(oc)
                    for o in range(D // P):
                        po = scratch.tile([P, QC], F32, tag="pacc")
                        for t in range(HC // P):
                            nc.tensor.matmul(
                                po,
                                wo_sb[t][:, o * P:(o + 1) * P],
                                an_c[t],
                                start=(t == 0), stop=(t == HC // P - 1))
                        ot = otp.tile([P, QC], F32, tag="ot")
                        nc.scalar.activation(
                            out=ot, in_=po,
                            func=mybir.ActivationFunctionType.Identity,
                            bias=bo_sb[:, o:o + 1], scale=1.0)
                        nc.sync.dma_start(
                            out=partT[o * P:(o + 1) * P, oc * QC:(oc + 1) * QC], in_=ot)

    with nc.Block() as block, nc.semaphore("cc_sem") as cc_sem, \
         nc.semaphore("dma_sem") as dma_sem:
        @block.gpsimd
        def _(gpsimd):
            gpsimd.collective_compute(
                "ReduceScatter", mybir.AluOpType.add,
                replica_groups=[[0, 1], [2, 3], [4, 5], [6, 7]],
                ins=[partT[:, :]], outs=[rs_out[:, :]],
            ).then_inc(cc_sem, 1)
            gpsimd.wait_ge(cc_sem, 1)
            gpsimd.dma_start(out=outTh[:, :], in_=rs_out[:, :]).then_inc(dma_sem, 16)
            gpsimd.wait_ge(dma_sem, 16)

    nc.compile()
    return nc


def _make_in_maps(x, Wq, bq, Wk, bk, Wv, bv, Wo, bo, mask):
    ref = np.tril(np.ones((L, L), dtype=np.int32))[None, None]
    assert np.array_equal(np.asarray(mask), ref), "mask must be causal"

    # m01 patterns for the 4 diagonal k-tiles of a q-chunk:
    # pattern_m[p, f] = 1 if p <= f - 128*m
    pf = np.arange(QC)[None, :] - np.arange(P)[:, None]
    m01 = np.concatenate(
        [(pf >= 128 * m).astype(np.float32) for m in range(4)], axis=1)

    in_maps = []
    for c in range(N_CORES):
        b, g = c // 2, c % 2
        cols = slice(HC * g, HC * g + HC)
        in_maps.append({
            "xT": np.ascontiguousarray(np.asarray(x[b]).T),
            "wq": np.ascontiguousarray(np.asarray(Wq)[:, cols]),
            "wk": np.ascontiguousarray(np.asarray(Wk)[:, cols]),
            "wv": np.ascontiguousarray(np.asarray(Wv)[:, cols]),
            "wo": np.ascontiguousarray(np.asarray(Wo)[cols, :]).astype(ml_dtypes.bfloat16),
            "bq": np.ascontiguousarray(np.asarray(bq)[cols].reshape(HC // P, P).T),
            "bk": np.ascontiguousarray(np.asarray(bk)[cols].reshape(HC // P, P).T),
            "bv": np.ascontiguousarray(
                np.broadcast_to(np.asarray(bv)[cols], (P, HC))),
            "bo": np.ascontiguousarray(
                (np.asarray(bo) / 2.0).reshape(D // P, P).T.astype(np.float32)),
            "m01": m01.astype(ml_dtypes.bfloat16),
        })
    return in_maps


def kernel(x, Wq, bq, Wk, bk, Wv, bv, Wo, bo, mask):
    global _NC, LAST_EXEC_NS
    if _NC is None:
        _NC = build_nc()
    in_maps = _make_in_maps(x, Wq, bq, Wk, bk, Wv, bv, Wo, bo, mask)
    r = run_bass_kernel_spmd(
        _NC, in_maps, core_ids=list(range(N_CORES)), trace=TRACE)
    LAST_EXEC_NS = r.exec_time_ns
    out = np.empty((B, L, D), dtype=np.float32)
    for b in range(B):
        outT = np.concatenate(
            [r.results[2 * b]["outTh"], r.results[2 * b + 1]["outTh"]], axis=0)
        out[b] = outT.T
    return out



# revision 8
# speedup vs baseline: 1.3552x; 1.3552x over previous
"""Multi-head causal attention on 8 Trainium2 NeuronCores.

Sharding: core c handles batch b = c // 2 and head-group g = c % 2
(8 of 16 heads, i.e. 512 of 1024 projection columns).  QKV projections,
attention and the output projection partial run per-core; the two cores
of a batch pair-ReduceScatter their partial outputs (pipelined per
512-seq chunk so the collective overlaps compute).

Everything on-device is computed in a transposed layout (seq on the
free dim) so no PE transposes are needed anywhere:
  xT [D, L] (host-pre-transposed, bf16) -> qT/kT [512, L] bf16
  -> S^T [keys, q] -> P^T = exp(S^T) (bf16) -> attn^T = (v|ones)^T @ P^T
  -> out^T = Wo^T @ attn_norm^T.  Host un-transposes the result.

v2 changes vs baseline:
  - all matmul operands bf16 (was f32r) -> cheaper LDWEIGHTS, less SBUF BW
  - causal diagonal tiles: S^T/exp computed only for the valid column
    suffix; the [128,128] diagonal block is triangle-masked with one DVE
    mul; dedicated prefix-zeroed pt tiles replace full-width mask muls
  - softmax 1/Z via reciprocal_approx_fast (was 3.4us/op DVE reciprocal)
  - output-projection bias-add on DVE (tensor_scalar_add), frees Scalar
  - partial outputs in bf16, pair-ReduceScatter per chunk *inside* the
    tile context -> collective overlaps compute instead of a 134us tail
"""

import sys, types

sys.path.insert(0, "/opt/trn_rl_repo")

# antenv.axon_hooks is missing in this image; inject it so trace=True can
# reach the NTFF profiling hook (used by test.py, off by default).
if "antenv.axon_hooks" not in sys.modules:
    _hook_mod = types.ModuleType("antenv.axon_hooks")
    _hook_mod._hook = None
    def _set_hook(h):
        _hook_mod._hook = h
    def _get_hook():
        return _hook_mod._hook
    _hook_mod.set_axon_ntff_profile_hook = _set_hook
    _hook_mod.get_axon_ntff_profile_hook = _get_hook
    sys.modules["antenv.axon_hooks"] = _hook_mod
    try:
        import antenv
        antenv.axon_hooks = _hook_mod
        from trn_agent_boot.trn_boot import _ntff_profile_via_ctypes
        _set_hook(_ntff_profile_via_ctypes("/opt/axon/libaxon_pjrt.so"))
    except Exception:
        pass

import numpy as np
import ml_dtypes
import concourse.bass as bass
import concourse.mybir as mybir
import concourse.tile as tile
from concourse import bacc
from concourse.bass_utils import run_bass_kernel_spmd

B, L, D, H = 4, 2048, 1024, 16
DH = 64
N_CORES = 8
NH = 8          # heads per core
HC = NH * DH    # 512 projection cols per core
QC = 512        # q-chunk
KT = 128        # k-tile
P = 128

F32 = mybir.dt.float32
BF16 = mybir.dt.bfloat16

TRACE = False
LAST_EXEC_NS = None
_NC = None


def build_nc(seq_len=L):
    Ls = seq_len
    NQC = Ls // QC
    NKT = Ls // KT
    NDS = D // P       # 8 contraction tiles for projections
    nc = bacc.Bacc()

    xT = nc.declare_dram_parameter("xT", [D, Ls], BF16, isOutput=False)
    wq = nc.declare_dram_parameter("wq", [D, HC], BF16, isOutput=False)
    wk = nc.declare_dram_parameter("wk", [D, HC], BF16, isOutput=False)
    wv = nc.declare_dram_parameter("wv", [D, HC], BF16, isOutput=False)
    wo = nc.declare_dram_parameter("wo", [HC, D], BF16, isOutput=False)
    bq = nc.declare_dram_parameter("bq", [P, HC // P], F32, isOutput=False)
    bk = nc.declare_dram_parameter("bk", [P, HC // P], F32, isOutput=False)
    bv = nc.declare_dram_parameter("bv", [P, HC], F32, isOutput=False)
    bo = nc.declare_dram_parameter("bo", [P, D // P], F32, isOutput=False)
    tri = nc.declare_dram_parameter("tri", [P, P], BF16, isOutput=False)
    outTh = nc.declare_dram_parameter("outTh", [D // 2, Ls], BF16, isOutput=True)

    scale = 1.0 / np.sqrt(np.float32(DH))

    from contextlib import ExitStack
    with nc.allow_low_precision(reason="bf16 matmuls by design; tol 2e-2"), \
         tile.TileContext(nc) as tc, ExitStack() as ctx:
        consts = ctx.enter_context(tc.tile_pool(name="consts", bufs=1))
        wpool = ctx.enter_context(tc.tile_pool(name="wpool", bufs=1))
        kvres = ctx.enter_context(tc.tile_pool(name="kvres", bufs=1))
        xtp = ctx.enter_context(tc.tile_pool(name="xtp", bufs=8))
        qtp = ctx.enter_context(tc.tile_pool(name="qtp", bufs=8))
        ptp = ctx.enter_context(tc.tile_pool(name="ptp", bufs=8))
        anp = ctx.enter_context(tc.tile_pool(name="anp", bufs=8))
        otp = ctx.enter_context(tc.tile_pool(name="otp", bufs=3))
        zrp = ctx.enter_context(tc.tile_pool(name="zrp", bufs=2))
        bzsb = ctx.enter_context(tc.tile_pool(name="bzsb", bufs=2))
        anodd = ctx.enter_context(tc.tile_pool(name="anodd", bufs=2))
        zdp = ctx.enter_context(tc.tile_pool(name="zdp", bufs=4, space="DRAM"))
        dramp = ctx.enter_context(tc.tile_pool(name="dramp", bufs=1, space="DRAM"))
        scratch = ctx.enter_context(tc.tile_pool(name="scratch", bufs=2, space="PSUM"))
        stp = ctx.enter_context(tc.tile_pool(name="stp", bufs=2, space="PSUM"))
        accp = ctx.enter_context(tc.tile_pool(name="accp", bufs=2, space="PSUM"))

        if True:
            # per-chunk bounce buffers for the pair-ReduceScatter (pool
            # tiles so the Tile framework tracks the DMA -> CC -> DMA deps)
            partT = [dramp.tile([D, QC], BF16, tag=f"partT{c}", name=f"partT{c}")
                     for c in range(NQC)]
            rs_out = [dramp.tile([D // 2, QC], BF16, tag=f"rs{c}", name=f"rs{c}")
                      for c in range(NQC)]

            # ---- constants ----
            bq_sb = consts.tile([P, HC // P], F32, tag="bq")
            bk_sb = consts.tile([P, HC // P], F32, tag="bk")
            bv_sb = consts.tile([P, HC], F32, tag="bv")
            bo_sb = consts.tile([P, D // P], F32, tag="bo")
            tri_sb = consts.tile([P, P], BF16, tag="tri")
            nc.sync.dma_start(out=bq_sb, in_=bq[:, :])
            nc.sync.dma_start(out=bk_sb, in_=bk[:, :])
            nc.sync.dma_start(out=bv_sb, in_=bv[:, :])
            nc.sync.dma_start(out=bo_sb, in_=bo[:, :])
            nc.sync.dma_start(out=tri_sb, in_=tri[:, :])

            # ---- weights resident ----
            wq_sb = [wpool.tile([P, HC], BF16, tag=f"wq{ds}", name=f"wq{ds}") for ds in range(NDS)]
            wk_sb = [wpool.tile([P, HC], BF16, tag=f"wk{ds}", name=f"wk{ds}") for ds in range(NDS)]
            wv_sb = [wpool.tile([P, HC], BF16, tag=f"wv{ds}", name=f"wv{ds}") for ds in range(NDS)]
            wo_sb = [wpool.tile([P, D], BF16, tag=f"wo{t}", name=f"wo{t}") for t in range(HC // P)]
            for ds in range(NDS):
                nc.sync.dma_start(out=wq_sb[ds], in_=wq[ds * P:(ds + 1) * P, :])
            for ds in range(NDS):
                nc.sync.dma_start(out=wk_sb[ds], in_=wk[ds * P:(ds + 1) * P, :])
                nc.sync.dma_start(out=wv_sb[ds], in_=wv[ds * P:(ds + 1) * P, :])
            for t in range(HC // P):
                nc.sync.dma_start(out=wo_sb[t], in_=wo[t * P:(t + 1) * P, :])

            # ---- resident kT and v ----
            kT_sb = [kvres.tile([P, Ls], BF16, tag=f"kT{t}", name=f"kT{t}") for t in range(HC // P)]
            # v: per key-tile [128, NH, 65] bf16; cols 0..63 = v, col 64 = ones
            # (the ones column makes the AV matmul emit softmax Z in row 64)
            v_sb = [kvres.tile([P, NH, 65], BF16, tag=f"v{kt}", name=f"v{kt}") for kt in range(NKT)]
            for kt in range(NKT):
                nc.vector.memset(v_sb[kt], 1.0)

            # ---- dedicated diagonal-P^T tiles: for diag k-tile m (= j-4c),
            # exp writes cols [128m, 512); the prefix [0, 128m) must read as
            # zero in the AV matmul, so it is zeroed ONCE here and never
            # written again. 2 buffers ping-pong across heads. ----
            ptd = [[kvres.tile([P, QC], BF16, tag=f"ptd{m}_{b_}", name=f"ptd{m}_{b_}")
                    for b_ in range(2)] for m in range(4)]
            for m in range(1, 4):
                for b_ in range(2):
                    nc.vector.memset(ptd[m][b_][:, 0:P * m], 0.0)

            # ---- per-chunk: projections for chunk s, then attention and
            # output projection for q-chunk c=s (causal => only needs k/v
            # from chunks <= s) ----
            xT_t = {}
            qT_t = {}
            attn_by_chunk = {}
            for s in range(NQC):
                for ds in range(NDS):
                    xt = xtp.tile([P, QC], BF16, tag="xT")
                    nc.sync.dma_start(
                        out=xt, in_=xT[ds * P:(ds + 1) * P, s * QC:(s + 1) * QC])
                    xT_t[(ds, s)] = xt

                for t in range(HC // P):
                    # qT tile [128 outcol, QC seq]
                    pq = scratch.tile([P, QC], F32, tag="pacc")
                    for ds in range(NDS):
                        nc.tensor.matmul(
                            pq,
                            wq_sb[ds][:, t * P:(t + 1) * P],
                            xT_t[(ds, s)],
                            start=(ds == 0), stop=(ds == NDS - 1))
                    qt = qtp.tile([P, QC], BF16, tag="qT")
                    nc.scalar.activation(
                        out=qt, in_=pq,
                        func=mybir.ActivationFunctionType.Identity,
                        bias=bq_sb[:, t:t + 1], scale=1.0)
                    qT_t[(t, s)] = qt

                    pk = scratch.tile([P, QC], F32, tag="pacc")
                    for ds in range(NDS):
                        nc.tensor.matmul(
                            pk,
                            wk_sb[ds][:, t * P:(t + 1) * P],
                            xT_t[(ds, s)],
                            start=(ds == 0), stop=(ds == NDS - 1))
                    nc.scalar.activation(
                        out=kT_sb[t][:, s * QC:(s + 1) * QC], in_=pk,
                        func=mybir.ActivationFunctionType.Identity,
                        bias=bk_sb[:, t:t + 1], scale=1.0)

                # v for the 4 key-tiles of this seq chunk
                for sub in range(QC // P):
                    kt = s * (QC // P) + sub
                    pv = scratch.tile([P, HC], F32, tag="pacc")
                    for ds in range(NDS):
                        nc.tensor.matmul(
                            pv,
                            xT_t[(ds, s)][:, sub * P:(sub + 1) * P],
                            wv_sb[ds],
                            start=(ds == 0), stop=(ds == NDS - 1))
                    nc.vector.tensor_add(
                        v_sb[kt][:, :, 0:64],
                        pv[:].rearrange("p (h d) -> p h d", h=NH),
                        bv_sb[:].rearrange("p (h d) -> p h d", h=NH))

                # ---- attention + output projection for q-chunk c = s ----
                c = s
                njt = min(4 * c + 4, NKT)     # causal: k-tiles 0..4c+3
                nfull = 4 * c                 # k-tiles fully below the diagonal
                attn_n = {}
                for t in range(HC // P):
                    an_t = anp.tile([P, QC], BF16, tag="an")
                    for par in range(2):
                        h = 2 * t + par
                        # S^T and P^T: full k-tile pairs, then the 4 diagonal
                        # k-tiles with causal column trimming
                        av_rhs = []
                        for g in range(nfull // 2):
                            st = stp.tile([P, 2 * QC], F32, tag="st")
                            for half in range(2):
                                j = 2 * g + half
                                nc.tensor.matmul(
                                    st[:, half * QC:(half + 1) * QC],
                                    kT_sb[t][par * DH:(par + 1) * DH,
                                             j * KT:(j + 1) * KT],
                                    qT_t[(t, c)][par * DH:(par + 1) * DH, :],
                                    start=True, stop=True)
                            pt = ptp.tile([P, 2 * QC], BF16, tag="pt")
                            nc.scalar.activation(
                                out=pt, in_=st,
                                func=mybir.ActivationFunctionType.Exp,
                                scale=float(scale))
                            av_rhs.append(pt[:, 0:QC])
                            av_rhs.append(pt[:, QC:2 * QC])
                        # diagonal k-tiles (2 per PSUM tile): compute only the
                        # valid column suffix [128m, 512), then triangle-mask
                        # the [128,128] diagonal block
                        for g in range(2):
                            st = stp.tile([P, 2 * QC], F32, tag="st")
                            for half in range(2):
                                m = 2 * g + half
                                j = nfull + m
                                lo = P * m
                                nc.tensor.matmul(
                                    st[:, half * QC + lo:half * QC + QC],
                                    kT_sb[t][par * DH:(par + 1) * DH,
                                             j * KT:(j + 1) * KT],
                                    qT_t[(t, c)][par * DH:(par + 1) * DH, lo:QC],
                                    start=True, stop=True)
                                pd = ptd[m][par]
                                nc.scalar.activation(
                                    out=pd[:, lo:QC],
                                    in_=st[:, half * QC + lo:half * QC + QC],
                                    func=mybir.ActivationFunctionType.Exp,
                                    scale=float(scale))
                                nc.vector.tensor_mul(
                                    pd[:, lo:lo + P], pd[:, lo:lo + P], tri_sb)
                                av_rhs.append(pd[:, 0:QC])
                        # AV with fused ones column -> rows 0..63 attn, row 64 = Z
                        acc = accp.tile([P, QC], F32, tag="acc")
                        for j in range(njt):
                            nc.tensor.matmul(
                                acc[0:65, :],
                                v_sb[j][:, h, :],
                                av_rhs[j],
                                start=(j == 0), stop=(j == njt - 1))
                        # normalization: zrec = 1/Z, broadcast over 64 rows via
                        # a DMA round-trip through DRAM (lane shift)
                        zrec = zrp.tile([P, QC], F32, tag="zrec")
                        nc.vector.reciprocal(
                            out=zrec[64:65, :], in_=acc[64:65, :])
                        bzs = bzsb.tile([P, QC], F32, tag="bzs")
                        zrow = zdp.tile([1, QC], F32, tag="zd", name="zrow")
                        nc.sync.dma_start(out=zrow, in_=zrec[64:65, :])
                        nc.sync.dma_start(out=bzs[0:DH, :].unsqueeze(1),
                                          in_=zrow.partition_broadcast(DH))
                        if par == 0:
                            nc.vector.tensor_mul(
                                an_t[0:DH, :], acc[0:DH, :], bzs[0:DH, :])
                        else:
                            an_o = anodd.tile([DH, QC], BF16, tag="anodd")
                            nc.vector.tensor_mul(
                                an_o, acc[0:DH, :], bzs[0:DH, :])
                            # shift to partitions 64..127 (DMA can cross lanes)
                            nc.sync.dma_start(out=an_t[DH:P, :], in_=an_o)
                    attn_n[t] = an_t

                attn_by_chunk[c] = attn_n
                # output projection, delayed one chunk so the softmax
                # normalization chain of chunk c overlaps proj matmuls of c+1;
                # each finished chunk immediately pair-ReduceScatters (bf16)
                # and writes its slice of the output, overlapping compute
                for oc in ([c - 1] if c + 1 < NQC else [c - 1, c]):
                    if oc < 0:
                        continue
                    an_c = attn_by_chunk.pop(oc)
                    for o in range(D // P):
                        po = scratch.tile([P, QC], F32, tag="pacc")
                        for t in range(HC // P):
                            nc.tensor.matmul(
                                po,
                                wo_sb[t][:, o * P:(o + 1) * P],
                                an_c[t],
                                start=(t == 0), stop=(t == HC // P - 1))
                        ot = otp.tile([P, QC], BF16, tag="ot")
                        nc.vector.tensor_scalar_add(ot, po, bo_sb[:, o:o + 1])
                        nc.sync.dma_start(
                            out=partT[oc][o * P:(o + 1) * P, :], in_=ot)
                    nc.gpsimd.collective_compute(
                        "ReduceScatter", mybir.AluOpType.add,
                        replica_groups=[[0, 1], [2, 3], [4, 5], [6, 7]],
                        ins=[partT[oc].opt()], outs=[rs_out[oc].opt()],
                    )
                    nc.sync.dma_start(
                        out=outTh[:, oc * QC:(oc + 1) * QC], in_=rs_out[oc][:, :])

    nc.compile()
    return nc


def _make_in_maps(x, Wq, bq, Wk, bk, Wv, bv, Wo, bo, mask):
    ref = np.tril(np.ones((L, L), dtype=np.int32))[None, None]
    assert np.array_equal(np.asarray(mask), ref), "mask must be causal"

    # triangle pattern for the diagonal [128,128] block: key p attends q f
    # iff p <= f
    tri = (np.arange(P)[:, None] <= np.arange(P)[None, :]).astype(np.float32)

    in_maps = []
    for c in range(N_CORES):
        b, g = c // 2, c % 2
        cols = slice(HC * g, HC * g + HC)
        in_maps.append({
            "xT": np.ascontiguousarray(np.asarray(x[b]).T).astype(ml_dtypes.bfloat16),
            "wq": np.ascontiguousarray(np.asarray(Wq)[:, cols]).astype(ml_dtypes.bfloat16),
            "wk": np.ascontiguousarray(np.asarray(Wk)[:, cols]).astype(ml_dtypes.bfloat16),
            "wv": np.ascontiguousarray(np.asarray(Wv)[:, cols]).astype(ml_dtypes.bfloat16),
            "wo": np.ascontiguousarray(np.asarray(Wo)[cols, :]).astype(ml_dtypes.bfloat16),
            "bq": np.ascontiguousarray(np.asarray(bq)[cols].reshape(HC // P, P).T),
            "bk": np.ascontiguousarray(np.asarray(bk)[cols].reshape(HC // P, P).T),
            "bv": np.ascontiguousarray(
                np.broadcast_to(np.asarray(bv)[cols], (P, HC))),
            "bo": np.ascontiguousarray(
                (np.asarray(bo) / 2.0).reshape(D // P, P).T.astype(np.float32)),
            "tri": tri.astype(ml_dtypes.bfloat16),
        })
    return in_maps


def kernel(x, Wq, bq, Wk, bk, Wv, bv, Wo, bo, mask):
    global _NC, LAST_EXEC_NS
    if _NC is None:
        _NC = build_nc()
    in_maps = _make_in_maps(x, Wq, bq, Wk, bk, Wv, bv, Wo, bo, mask)
    r = run_bass_kernel_spmd(
        _NC, in_maps, core_ids=list(range(N_CORES)), trace=TRACE)
    LAST_EXEC_NS = r.exec_time_ns
    out = np.empty((B, L, D), dtype=np.float32)
    for b in range(B):
        outT = np.concatenate(
            [r.results[2 * b]["outTh"].astype(np.float32),
             r.results[2 * b + 1]["outTh"].astype(np.float32)], axis=0)
        out[b] = outT.T
    return out


# revision 17
# speedup vs baseline: 1.4878x; 1.0978x over previous
"""Multi-head causal attention on 8 Trainium2 NeuronCores.

Sharding: core c handles batch b = c // 2 and head-group g = c % 2
(8 of 16 heads, i.e. 512 of 1024 projection columns).  QKV projections,
attention and the output projection partial run per-core; the two cores
of a batch pair-ReduceScatter their partial outputs (pipelined per
512-seq chunk so the collective overlaps compute).

Everything on-device is computed in a transposed layout (seq on the
free dim) so no PE transposes are needed anywhere:
  xT [D, L] (host-pre-transposed, bf16) -> qT/kT [512, L] bf16
  -> S^T [keys, q] -> P^T = exp(S^T) (bf16) -> attn^T = (v|ones)^T @ P^T
  -> out^T = Wo^T @ attn_norm^T.  Host un-transposes the result.

v2 changes vs baseline:
  - all matmul operands bf16 (was f32r) -> cheaper LDWEIGHTS, less SBUF BW
  - causal diagonal tiles: S^T/exp computed only for the valid column
    suffix; the [128,128] diagonal block is triangle-masked with one DVE
    mul; dedicated prefix-zeroed pt tiles replace full-width mask muls
  - softmax 1/Z via reciprocal_approx_fast (was 3.4us/op DVE reciprocal)
  - output-projection bias-add on DVE (tensor_scalar_add), frees Scalar
  - partial outputs in bf16, pair-ReduceScatter per chunk *inside* the
    tile context -> collective overlaps compute instead of a 134us tail
"""

import sys, types

sys.path.insert(0, "/opt/trn_rl_repo")

# antenv.axon_hooks is missing in this image; inject it so trace=True can
# reach the NTFF profiling hook (used by test.py, off by default).
if "antenv.axon_hooks" not in sys.modules:
    _hook_mod = types.ModuleType("antenv.axon_hooks")
    _hook_mod._hook = None
    def _set_hook(h):
        _hook_mod._hook = h
    def _get_hook():
        return _hook_mod._hook
    _hook_mod.set_axon_ntff_profile_hook = _set_hook
    _hook_mod.get_axon_ntff_profile_hook = _get_hook
    sys.modules["antenv.axon_hooks"] = _hook_mod
    try:
        import antenv
        antenv.axon_hooks = _hook_mod
        from trn_agent_boot.trn_boot import _ntff_profile_via_ctypes
        _set_hook(_ntff_profile_via_ctypes("/opt/axon/libaxon_pjrt.so"))
    except Exception:
        pass

import numpy as np
import ml_dtypes
import concourse.bass as bass
import concourse.mybir as mybir
import concourse.tile as tile
from concourse import bacc
from concourse.bass_utils import run_bass_kernel_spmd

B, L, D, H = 4, 2048, 1024, 16
DH = 64
N_CORES = 8
NH = 8          # heads per core
HC = NH * DH    # 512 projection cols per core
QC = 512        # q-chunk
KT = 128        # k-tile
P = 128

F32 = mybir.dt.float32
BF16 = mybir.dt.bfloat16

TRACE = False
LAST_EXEC_NS = None
_NC = None


def build_nc(seq_len=L):
    Ls = seq_len
    NQC = Ls // QC
    NKT = Ls // KT
    NDS = D // P       # 8 contraction tiles for projections
    nc = bacc.Bacc()

    xT = nc.declare_dram_parameter("xT", [D, Ls], BF16, isOutput=False)
    wq = nc.declare_dram_parameter("wq", [D, HC], BF16, isOutput=False)
    wk = nc.declare_dram_parameter("wk", [D, HC], BF16, isOutput=False)
    wv = nc.declare_dram_parameter("wv", [D, HC], BF16, isOutput=False)
    wo = nc.declare_dram_parameter("wo", [HC, D], BF16, isOutput=False)
    bq = nc.declare_dram_parameter("bq", [P, HC // P], F32, isOutput=False)
    bk = nc.declare_dram_parameter("bk", [P, HC // P], F32, isOutput=False)
    bv = nc.declare_dram_parameter("bv", [P, HC], F32, isOutput=False)
    bo = nc.declare_dram_parameter("bo", [P, D // P], F32, isOutput=False)
    tri = nc.declare_dram_parameter("tri", [P, P], BF16, isOutput=False)
    outTh = nc.declare_dram_parameter("outTh", [D // 2, Ls], BF16, isOutput=True)

    scale = 1.0 / np.sqrt(np.float32(DH))

    from contextlib import ExitStack
    with nc.allow_low_precision(reason="bf16 matmuls by design; tol 2e-2"), \
         tile.TileContext(nc) as tc, ExitStack() as ctx:
        consts = ctx.enter_context(tc.tile_pool(name="consts", bufs=1))
        wpool = ctx.enter_context(tc.tile_pool(name="wpool", bufs=1))
        kvres = ctx.enter_context(tc.tile_pool(name="kvres", bufs=1))
        xtp = ctx.enter_context(tc.tile_pool(name="xtp", bufs=8))
        qtp = ctx.enter_context(tc.tile_pool(name="qtp", bufs=8))
        ptp = ctx.enter_context(tc.tile_pool(name="ptp", bufs=8))
        anp = ctx.enter_context(tc.tile_pool(name="anp", bufs=8))
        otp = ctx.enter_context(tc.tile_pool(name="otp", bufs=3))
        zrp = ctx.enter_context(tc.tile_pool(name="zrp", bufs=2))
        bzsb = ctx.enter_context(tc.tile_pool(name="bzsb", bufs=2))
        anodd = ctx.enter_context(tc.tile_pool(name="anodd", bufs=2))
        zdp = ctx.enter_context(tc.tile_pool(name="zdp", bufs=4, space="DRAM"))
        dramp = ctx.enter_context(tc.tile_pool(name="dramp", bufs=1, space="DRAM"))
        scratch = ctx.enter_context(tc.tile_pool(name="scratch", bufs=2, space="PSUM"))
        stp = ctx.enter_context(tc.tile_pool(name="stp", bufs=2, space="PSUM"))
        accp = ctx.enter_context(tc.tile_pool(name="accp", bufs=2, space="PSUM"))

        if True:
            # per-chunk bounce buffers for the pair-ReduceScatter (pool
            # tiles so the Tile framework tracks the DMA -> CC -> DMA deps)
            partT = [dramp.tile([D, QC], BF16, tag=f"partT{c}", name=f"partT{c}")
                     for c in range(NQC)]
            rs_out = [dramp.tile([D // 2, QC], BF16, tag=f"rs{c}", name=f"rs{c}")
                      for c in range(NQC)]

            # ---- constants ----
            bq_sb = consts.tile([P, HC // P], F32, tag="bq")
            bk_sb = consts.tile([P, HC // P], F32, tag="bk")
            bv_sb = consts.tile([P, HC], F32, tag="bv")
            bo_sb = consts.tile([P, D // P], F32, tag="bo")
            tri_sb = consts.tile([P, P], BF16, tag="tri")
            nc.sync.dma_start(out=bq_sb, in_=bq[:, :])
            nc.sync.dma_start(out=bk_sb, in_=bk[:, :])
            nc.sync.dma_start(out=bv_sb, in_=bv[:, :])
            nc.sync.dma_start(out=bo_sb, in_=bo[:, :])
            nc.sync.dma_start(out=tri_sb, in_=tri[:, :])

            # ---- weights resident ----
            wq_sb = [wpool.tile([P, HC], BF16, tag=f"wq{ds}", name=f"wq{ds}") for ds in range(NDS)]
            wk_sb = [wpool.tile([P, HC], BF16, tag=f"wk{ds}", name=f"wk{ds}") for ds in range(NDS)]
            wv_sb = [wpool.tile([P, HC], BF16, tag=f"wv{ds}", name=f"wv{ds}") for ds in range(NDS)]
            wo_sb = [wpool.tile([P, D], BF16, tag=f"wo{t}", name=f"wo{t}") for t in range(HC // P)]
            # wq on the sync queue (needed first); wk/wv/wo on other
            # engines' DMA queues so the rings run in parallel and the
            # first Q matmul isn't stuck behind 4 MB of weight traffic
            for ds in range(NDS):
                nc.sync.dma_start(out=wq_sb[ds], in_=wq[ds * P:(ds + 1) * P, :])
            for ds in range(NDS):
                nc.scalar.dma_start(out=wk_sb[ds], in_=wk[ds * P:(ds + 1) * P, :])
                nc.gpsimd.dma_start(out=wv_sb[ds], in_=wv[ds * P:(ds + 1) * P, :])
            for t in range(HC // P):
                nc.gpsimd.dma_start(out=wo_sb[t], in_=wo[t * P:(t + 1) * P, :])

            # ---- resident kT and v ----
            kT_sb = [kvres.tile([P, Ls], BF16, tag=f"kT{t}", name=f"kT{t}") for t in range(HC // P)]
            # v: per key-tile [128, NH, 65] bf16; cols 0..63 = v, col 64 = ones
            # (the ones column makes the AV matmul emit softmax Z in row 64)
            v_sb = [kvres.tile([P, NH, 65], BF16, tag=f"v{kt}", name=f"v{kt}") for kt in range(NKT)]
            for kt in range(NKT):
                # only the ones-column needs initialization; cols 0..63 are
                # fully overwritten by the v-projection bias add
                nc.vector.memset(v_sb[kt][:, :, 64:65], 1.0)

            # ---- dedicated diagonal-P^T tiles: for diag k-tile m (= j-4c),
            # exp writes cols [128m, 512); the prefix [0, 128m) must read as
            # zero in the AV matmul, so it is zeroed ONCE here and never
            # written again. 2 buffers ping-pong across heads. ----
            ptd = [[kvres.tile([P, QC], BF16, tag=f"ptd{m}_{b_}", name=f"ptd{m}_{b_}")
                    for b_ in range(2)] for m in range(4)]
            for m in range(1, 4):
                for b_ in range(2):
                    nc.vector.memset(ptd[m][b_][:, 0:P * m], 0.0)

            # ---- per-chunk: projections for chunk s, then attention and
            # output projection for q-chunk c=s (causal => only needs k/v
            # from chunks <= s) ----
            xT_t = {}
            qT_t = {}
            attn_by_chunk = {}
            for s in range(NQC):
                for ds in range(NDS):
                    xt = xtp.tile([P, QC], BF16, tag="xT")
                    nc.sync.dma_start(
                        out=xt, in_=xT[ds * P:(ds + 1) * P, s * QC:(s + 1) * QC])
                    xT_t[(ds, s)] = xt

                for t in range(HC // P):
                    # qT tile [128 outcol, QC seq]
                    pq = scratch.tile([P, QC], F32, tag="pacc")
                    for ds in range(NDS):
                        nc.tensor.matmul(
                            pq,
                            wq_sb[ds][:, t * P:(t + 1) * P],
                            xT_t[(ds, s)],
                            start=(ds == 0), stop=(ds == NDS - 1))
                    qt = qtp.tile([P, QC], BF16, tag="qT")
                    nc.vector.tensor_scalar_add(qt, pq, bq_sb[:, t:t + 1])
                    qT_t[(t, s)] = qt

                    pk = scratch.tile([P, QC], F32, tag="pacc")
                    for ds in range(NDS):
                        nc.tensor.matmul(
                            pk,
                            wk_sb[ds][:, t * P:(t + 1) * P],
                            xT_t[(ds, s)],
                            start=(ds == 0), stop=(ds == NDS - 1))
                    nc.vector.tensor_scalar_add(
                        kT_sb[t][:, s * QC:(s + 1) * QC], pk, bk_sb[:, t:t + 1])

                # v for the 4 key-tiles of this seq chunk
                for sub in range(QC // P):
                    kt = s * (QC // P) + sub
                    pv = scratch.tile([P, HC], F32, tag="pacc")
                    for ds in range(NDS):
                        nc.tensor.matmul(
                            pv,
                            xT_t[(ds, s)][:, sub * P:(sub + 1) * P],
                            wv_sb[ds],
                            start=(ds == 0), stop=(ds == NDS - 1))
                    # gpsimd cannot read PSUM; this stays on DVE
                    nc.vector.tensor_add(
                        v_sb[kt][:, :, 0:64],
                        pv[:].rearrange("p (h d) -> p h d", h=NH),
                        bv_sb[:].rearrange("p (h d) -> p h d", h=NH))

                # ---- attention + output projection for q-chunk c = s ----
                c = s
                njt = min(4 * c + 4, NKT)     # causal: k-tiles 0..4c+3
                nfull = 4 * c                 # k-tiles fully below the diagonal
                attn_n = {}
                for t in range(HC // P):
                    an_t = anp.tile([P, QC], BF16, tag="an")
                    for par in range(2):
                        h = 2 * t + par
                        # S^T and P^T: full k-tile pairs, then the 4 diagonal
                        # k-tiles with causal column trimming
                        av_full = []
                        av_diag = []
                        for g in range(nfull // 2):
                            st = stp.tile([P, 2 * QC], F32, tag="st")
                            for half in range(2):
                                j = 2 * g + half
                                nc.tensor.matmul(
                                    st[:, half * QC:(half + 1) * QC],
                                    kT_sb[t][par * DH:(par + 1) * DH,
                                             j * KT:(j + 1) * KT],
                                    qT_t[(t, c)][par * DH:(par + 1) * DH, :],
                                    start=True, stop=True)
                            pt = ptp.tile([P, 2 * QC], BF16, tag="pt")
                            nc.scalar.activation(
                                out=pt, in_=st,
                                func=mybir.ActivationFunctionType.Exp,
                                scale=float(scale))
                            av_full.append((2 * g, pt[:, 0:QC]))
                            av_full.append((2 * g + 1, pt[:, QC:2 * QC]))
                        # diagonal k-tiles (2 per PSUM tile): compute only the
                        # valid column suffix [128m, 512), then triangle-mask
                        # the [128,128] diagonal block
                        for g in range(2):
                            st = stp.tile([P, 2 * QC], F32, tag="st")
                            for half in range(2):
                                m = 2 * g + half
                                j = nfull + m
                                lo = P * m
                                nc.tensor.matmul(
                                    st[:, half * QC + lo:half * QC + QC],
                                    kT_sb[t][par * DH:(par + 1) * DH,
                                             j * KT:(j + 1) * KT],
                                    qT_t[(t, c)][par * DH:(par + 1) * DH, lo:QC],
                                    start=True, stop=True)
                                pd = ptd[m][par]
                                nc.scalar.activation(
                                    out=pd[:, lo:QC],
                                    in_=st[:, half * QC + lo:half * QC + QC],
                                    func=mybir.ActivationFunctionType.Exp,
                                    scale=float(scale))
                                nc.gpsimd.tensor_mul(
                                    pd[:, lo:lo + P], pd[:, lo:lo + P], tri_sb)
                                av_diag.append((j, pd, lo))
                        # AV with fused ones column -> rows 0..63 attn, row
                        # 64 = Z.  Diagonal tiles stream first (only their
                        # valid column suffix when a full tile follows to
                        # close the accumulation group); full tiles last.
                        acc = accp.tile([P, QC], F32, tag="acc")
                        for idx, (j, pd, lo) in enumerate(av_diag):
                            if nfull == 0:
                                lo = 0      # no full tile after: keep the
                                            # group full-width (prefix is 0)
                            nc.tensor.matmul(
                                acc[0:65, lo:QC],
                                v_sb[j][:, h, :],
                                pd[:, lo:QC],
                                start=(idx == 0),
                                stop=(nfull == 0 and idx == 3))
                        for idx, (j, rhs) in enumerate(av_full):
                            nc.tensor.matmul(
                                acc[0:65, :],
                                v_sb[j][:, h, :],
                                rhs,
                                start=False, stop=(idx == nfull - 1))
                        # normalization: broadcast Z over 64 rows via a DMA
                        # round-trip through DRAM (lane shift), then a fast
                        # approximate reciprocal on partitions 0..63
                        zraw = zrp.tile([P, QC], F32, tag="zraw")
                        bzs = bzsb.tile([P, QC], F32, tag="bzs")
                        zrow = zdp.tile([1, QC], F32, tag="zd", name="zrow")
                        # DMA cannot read PSUM: bounce Z through SBUF
                        nc.vector.tensor_copy(zraw[64:65, :], acc[64:65, :])
                        nc.sync.dma_start(out=zrow, in_=zraw[64:65, :])
                        nc.sync.dma_start(out=zraw[0:DH, :].unsqueeze(1),
                                          in_=zrow.partition_broadcast(DH))
                        nc.vector.reciprocal_approx_fast(
                            out=bzs[0:DH, :], in_=zraw[0:DH, :])
                        if par == 0:
                            nc.vector.tensor_mul(
                                an_t[0:DH, :], acc[0:DH, :], bzs[0:DH, :])
                        else:
                            an_o = anodd.tile([DH, QC], BF16, tag="anodd")
                            nc.vector.tensor_mul(
                                an_o, acc[0:DH, :], bzs[0:DH, :])
                            # shift to partitions 64..127 (DMA can cross lanes)
                            nc.sync.dma_start(out=an_t[DH:P, :], in_=an_o)
                    attn_n[t] = an_t

                attn_by_chunk[c] = attn_n
                # output projection, delayed one chunk so the softmax
                # normalization chain of chunk c overlaps proj matmuls of c+1;
                # each finished chunk immediately pair-ReduceScatters (bf16)
                # and writes its slice of the output, overlapping compute
                for oc in ([c - 1] if c + 1 < NQC else [c - 1, c]):
                    if oc < 0:
                        continue
                    an_c = attn_by_chunk.pop(oc)
                    for o in range(D // P):
                        po = scratch.tile([P, QC], F32, tag="pacc")
                        for t in range(HC // P):
                            nc.tensor.matmul(
                                po,
                                wo_sb[t][:, o * P:(o + 1) * P],
                                an_c[t],
                                start=(t == 0), stop=(t == HC // P - 1))
                        ot = otp.tile([P, QC], BF16, tag="ot")
                        nc.vector.tensor_scalar_add(ot, po, bo_sb[:, o:o + 1])
                        nc.sync.dma_start(
                            out=partT[oc][o * P:(o + 1) * P, :], in_=ot)
                    nc.gpsimd.collective_compute(
                        "ReduceScatter", mybir.AluOpType.add,
                        replica_groups=[[0, 1], [2, 3], [4, 5], [6, 7]],
                        ins=[partT[oc].opt()], outs=[rs_out[oc].opt()],
                    )
                    nc.sync.dma_start(
                        out=outTh[:, oc * QC:(oc + 1) * QC], in_=rs_out[oc][:, :])

    nc.compile()
    return nc


def _make_in_maps(x, Wq, bq, Wk, bk, Wv, bv, Wo, bo, mask):
    ref = np.tril(np.ones((L, L), dtype=np.int32))[None, None]
    assert np.array_equal(np.asarray(mask), ref), "mask must be causal"

    # triangle pattern for the diagonal [128,128] block: key p attends q f
    # iff p <= f
    tri = (np.arange(P)[:, None] <= np.arange(P)[None, :]).astype(np.float32)

    in_maps = []
    for c in range(N_CORES):
        b, g = c // 2, c % 2
        cols = slice(HC * g, HC * g + HC)
        in_maps.append({
            "xT": np.ascontiguousarray(np.asarray(x[b]).T).astype(ml_dtypes.bfloat16),
            "wq": np.ascontiguousarray(np.asarray(Wq)[:, cols]).astype(ml_dtypes.bfloat16),
            "wk": np.ascontiguousarray(np.asarray(Wk)[:, cols]).astype(ml_dtypes.bfloat16),
            "wv": np.ascontiguousarray(np.asarray(Wv)[:, cols]).astype(ml_dtypes.bfloat16),
            "wo": np.ascontiguousarray(np.asarray(Wo)[cols, :]).astype(ml_dtypes.bfloat16),
            "bq": np.ascontiguousarray(np.asarray(bq)[cols].reshape(HC // P, P).T),
            "bk": np.ascontiguousarray(np.asarray(bk)[cols].reshape(HC // P, P).T),
            "bv": np.ascontiguousarray(
                np.broadcast_to(np.asarray(bv)[cols], (P, HC))),
            "bo": np.ascontiguousarray(
                (np.asarray(bo) / 2.0).reshape(D // P, P).T.astype(np.float32)),
            "tri": tri.astype(ml_dtypes.bfloat16),
        })
    return in_maps


def kernel(x, Wq, bq, Wk, bk, Wv, bv, Wo, bo, mask):
    global _NC, LAST_EXEC_NS
    if _NC is None:
        _NC = build_nc()
    in_maps = _make_in_maps(x, Wq, bq, Wk, bk, Wv, bv, Wo, bo, mask)
    r = run_bass_kernel_spmd(
        _NC, in_maps, core_ids=list(range(N_CORES)), trace=TRACE)
    LAST_EXEC_NS = r.exec_time_ns
    out = np.empty((B, L, D), dtype=np.float32)
    for b in range(B):
        outT = np.concatenate(
            [r.results[2 * b]["outTh"].astype(np.float32),
             r.results[2 * b + 1]["outTh"].astype(np.float32)], axis=0)
        out[b] = outT.T
    return out


# revision 22
# speedup vs baseline: 1.5628x; 1.0505x over previous
"""Multi-head causal attention on 8 Trainium2 NeuronCores.

Sharding: core c handles batch b = c // 2 and head-group g = c % 2
(8 of 16 heads, i.e. 512 of 1024 projection columns).  QKV projections,
attention and the output projection partial run per-core; the two cores
of a batch pair-ReduceScatter their partial outputs (pipelined per
512-seq chunk so the collective overlaps compute).

Everything on-device is computed in a transposed layout (seq on the
free dim) so no PE transposes are needed anywhere:
  xT [D, L] (host-pre-transposed, bf16) -> qT/kT [512, L] bf16
  -> S^T [keys, q] -> P^T = exp(S^T) (bf16) -> attn^T = (v|ones)^T @ P^T
  -> out^T = Wo^T @ attn_norm^T.  Host un-transposes the result.

v2 changes vs baseline:
  - all matmul operands bf16 (was f32r) -> cheaper LDWEIGHTS, less SBUF BW
  - causal diagonal tiles: S^T/exp computed only for the valid column
    suffix; the [128,128] diagonal block is triangle-masked with one DVE
    mul; dedicated prefix-zeroed pt tiles replace full-width mask muls
  - softmax 1/Z via reciprocal_approx_fast (was 3.4us/op DVE reciprocal)
  - output-projection bias-add on DVE (tensor_scalar_add), frees Scalar
  - partial outputs in bf16, pair-ReduceScatter per chunk *inside* the
    tile context -> collective overlaps compute instead of a 134us tail
"""

import sys, types

sys.path.insert(0, "/opt/trn_rl_repo")

# antenv.axon_hooks is missing in this image; inject it so trace=True can
# reach the NTFF profiling hook (used by test.py, off by default).
if "antenv.axon_hooks" not in sys.modules:
    _hook_mod = types.ModuleType("antenv.axon_hooks")
    _hook_mod._hook = None
    def _set_hook(h):
        _hook_mod._hook = h
    def _get_hook():
        return _hook_mod._hook
    _hook_mod.set_axon_ntff_profile_hook = _set_hook
    _hook_mod.get_axon_ntff_profile_hook = _get_hook
    sys.modules["antenv.axon_hooks"] = _hook_mod
    try:
        import antenv
        antenv.axon_hooks = _hook_mod
        from trn_agent_boot.trn_boot import _ntff_profile_via_ctypes
        _set_hook(_ntff_profile_via_ctypes("/opt/axon/libaxon_pjrt.so"))
    except Exception:
        pass

import numpy as np
import ml_dtypes
import concourse.bass as bass
import concourse.mybir as mybir
import concourse.tile as tile
from concourse import bacc
from concourse.bass_utils import run_bass_kernel_spmd

B, L, D, H = 4, 2048, 1024, 16
DH = 64
N_CORES = 8
NH = 8          # heads per core
HC = NH * DH    # 512 projection cols per core
QC = 512        # q-chunk
KT = 128        # k-tile
P = 128

F32 = mybir.dt.float32
BF16 = mybir.dt.bfloat16

TRACE = False
LAST_EXEC_NS = None
_NC = None


def build_nc(seq_len=L):
    Ls = seq_len
    NQC = Ls // QC
    NKT = Ls // KT
    NDS = D // P       # 8 contraction tiles for projections
    nc = bacc.Bacc()

    xT = nc.declare_dram_parameter("xT", [D, Ls], BF16, isOutput=False)
    wq = nc.declare_dram_parameter("wq", [D, HC], BF16, isOutput=False)
    wk = nc.declare_dram_parameter("wk", [D, HC], BF16, isOutput=False)
    wv = nc.declare_dram_parameter("wv", [D, HC], BF16, isOutput=False)
    wo = nc.declare_dram_parameter("wo", [HC, D], BF16, isOutput=False)
    bq = nc.declare_dram_parameter("bq", [P, HC // P], F32, isOutput=False)
    bk = nc.declare_dram_parameter("bk", [P, HC // P], F32, isOutput=False)
    bv = nc.declare_dram_parameter("bv", [P, HC], F32, isOutput=False)
    bo = nc.declare_dram_parameter("bo", [P, D // P], F32, isOutput=False)
    tri = nc.declare_dram_parameter("tri", [P, P], BF16, isOutput=False)
    outTh = nc.declare_dram_parameter("outTh", [D // 2, Ls], BF16, isOutput=True)

    scale = 1.0 / np.sqrt(np.float32(DH))

    from contextlib import ExitStack
    with nc.allow_low_precision(reason="bf16 matmuls by design; tol 2e-2"), \
         tile.TileContext(nc) as tc, ExitStack() as ctx:
        consts = ctx.enter_context(tc.tile_pool(name="consts", bufs=1))
        wpool = ctx.enter_context(tc.tile_pool(name="wpool", bufs=1))
        kvres = ctx.enter_context(tc.tile_pool(name="kvres", bufs=1))
        xtp = ctx.enter_context(tc.tile_pool(name="xtp", bufs=8))
        qtp = ctx.enter_context(tc.tile_pool(name="qtp", bufs=8))
        ptp = ctx.enter_context(tc.tile_pool(name="ptp", bufs=8))
        anp = ctx.enter_context(tc.tile_pool(name="anp", bufs=8))
        otp = ctx.enter_context(tc.tile_pool(name="otp", bufs=2))
        zrp = ctx.enter_context(tc.tile_pool(name="zrp", bufs=2))
        bzsb = ctx.enter_context(tc.tile_pool(name="bzsb", bufs=2))
        anodd = ctx.enter_context(tc.tile_pool(name="anodd", bufs=2))
        zdp = ctx.enter_context(tc.tile_pool(name="zdp", bufs=4, space="DRAM"))
        dramp = ctx.enter_context(tc.tile_pool(name="dramp", bufs=1, space="DRAM"))
        scratch = ctx.enter_context(tc.tile_pool(name="scratch", bufs=2, space="PSUM"))
        stp = ctx.enter_context(tc.tile_pool(name="stp", bufs=2, space="PSUM"))
        accp = ctx.enter_context(tc.tile_pool(name="accp", bufs=2, space="PSUM"))

        if True:
            # per-chunk bounce buffers for the pair-ReduceScatter (pool
            # tiles so the Tile framework tracks the DMA -> CC -> DMA deps)
            partT = [dramp.tile([D, QC], BF16, tag=f"partT{c}", name=f"partT{c}")
                     for c in range(NQC)]
            rs_out = [dramp.tile([D // 2, QC], BF16, tag=f"rs{c}", name=f"rs{c}")
                      for c in range(NQC)]

            # ---- constants ----
            bq_sb = consts.tile([P, HC // P], F32, tag="bq")
            bk_sb = consts.tile([P, HC // P], F32, tag="bk")
            bv_sb = consts.tile([P, HC], F32, tag="bv")
            bo_sb = consts.tile([P, D // P], F32, tag="bo")
            tri_sb = consts.tile([P, P], BF16, tag="tri")
            nc.sync.dma_start(out=bq_sb, in_=bq[:, :])
            nc.sync.dma_start(out=bk_sb, in_=bk[:, :])
            nc.sync.dma_start(out=bv_sb, in_=bv[:, :])
            nc.sync.dma_start(out=bo_sb, in_=bo[:, :])
            nc.sync.dma_start(out=tri_sb, in_=tri[:, :])

            # ---- weights resident ----
            wq_sb = [wpool.tile([P, HC], BF16, tag=f"wq{ds}", name=f"wq{ds}") for ds in range(NDS)]
            wk_sb = [wpool.tile([P, HC], BF16, tag=f"wk{ds}", name=f"wk{ds}") for ds in range(NDS)]
            wv_sb = [wpool.tile([P, HC], BF16, tag=f"wv{ds}", name=f"wv{ds}") for ds in range(NDS)]
            wo_sb = [wpool.tile([P, D], BF16, tag=f"wo{t}", name=f"wo{t}") for t in range(HC // P)]
            # wq + chunk-0 xT interleaved on the sync queue (both needed for
            # the first Q matmuls); wk/wv/wo on other engines' DMA queues so
            # the rings run in parallel and the first matmul isn't stuck
            # behind 4 MB of weight traffic
            xT_t = {}
            for ds in range(NDS):
                nc.sync.dma_start(out=wq_sb[ds], in_=wq[ds * P:(ds + 1) * P, :])
                xt = xtp.tile([P, QC], BF16, tag="xT")
                nc.sync.dma_start(out=xt, in_=xT[ds * P:(ds + 1) * P, 0:QC])
                xT_t[(ds, 0)] = xt
            for ds in range(NDS):
                nc.scalar.dma_start(out=wk_sb[ds], in_=wk[ds * P:(ds + 1) * P, :])
                nc.gpsimd.dma_start(out=wv_sb[ds], in_=wv[ds * P:(ds + 1) * P, :])
            for t in range(HC // P):
                nc.gpsimd.dma_start(out=wo_sb[t], in_=wo[t * P:(t + 1) * P, :])

            # ---- resident kT and v ----
            kT_sb = [kvres.tile([P, Ls], BF16, tag=f"kT{t}", name=f"kT{t}") for t in range(HC // P)]
            # v: per key-tile [128, NH, 65] bf16; cols 0..63 = v, col 64 = ones
            # (the ones column makes the AV matmul emit softmax Z in row 64)
            v_sb = [kvres.tile([P, NH, 65], BF16, tag=f"v{kt}", name=f"v{kt}") for kt in range(NKT)]
            for kt in range(NKT):
                # only the ones-column needs initialization; cols 0..63 are
                # fully overwritten by the v-projection bias add
                nc.vector.memset(v_sb[kt][:, :, 64:65], 1.0)

            # ---- dedicated diagonal-P^T tiles: for diag k-tile m (= j-4c),
            # exp writes cols [128m, 512); the prefix [0, 128m) must read as
            # zero in the AV matmul, so it is zeroed ONCE here and never
            # written again. 2 buffers ping-pong across heads. ----
            ptd = [[kvres.tile([P, QC], BF16, tag=f"ptd{m}_{b_}", name=f"ptd{m}_{b_}")
                    for b_ in range(2)] for m in range(4)]
            for m in range(1, 4):
                for b_ in range(2):
                    nc.vector.memset(ptd[m][b_][:, 0:P * m], 0.0)

            # ---- per-chunk: projections for chunk s, then attention and
            # output projection for q-chunk c=s (causal => only needs k/v
            # from chunks <= s) ----
            qT_t = {}
            attn_by_chunk = {}
            for s in range(NQC):
                for ds in range(NDS):
                    if (ds, s) in xT_t:
                        continue    # chunk 0 prefetched with the weights
                    xt = xtp.tile([P, QC], BF16, tag="xT")
                    nc.sync.dma_start(
                        out=xt, in_=xT[ds * P:(ds + 1) * P, s * QC:(s + 1) * QC])
                    xT_t[(ds, s)] = xt

                for t in range(HC // P):
                    # qT tile [128 outcol, QC seq]
                    pq = scratch.tile([P, QC], F32, tag="pacc")
                    for ds in range(NDS):
                        nc.tensor.matmul(
                            pq,
                            wq_sb[ds][:, t * P:(t + 1) * P],
                            xT_t[(ds, s)],
                            start=(ds == 0), stop=(ds == NDS - 1))
                    qt = qtp.tile([P, QC], BF16, tag="qT")
                    nc.vector.tensor_scalar_add(qt, pq, bq_sb[:, t:t + 1])
                    qT_t[(t, s)] = qt

                    pk = scratch.tile([P, QC], F32, tag="pacc")
                    for ds in range(NDS):
                        nc.tensor.matmul(
                            pk,
                            wk_sb[ds][:, t * P:(t + 1) * P],
                            xT_t[(ds, s)],
                            start=(ds == 0), stop=(ds == NDS - 1))
                    nc.vector.tensor_scalar_add(
                        kT_sb[t][:, s * QC:(s + 1) * QC], pk, bk_sb[:, t:t + 1])

                # v for the 4 key-tiles of this seq chunk
                for sub in range(QC // P):
                    kt = s * (QC // P) + sub
                    pv = scratch.tile([P, HC], F32, tag="pacc")
                    for ds in range(NDS):
                        nc.tensor.matmul(
                            pv,
                            xT_t[(ds, s)][:, sub * P:(sub + 1) * P],
                            wv_sb[ds],
                            start=(ds == 0), stop=(ds == NDS - 1))
                    # gpsimd cannot read PSUM; this stays on DVE
                    nc.vector.tensor_add(
                        v_sb[kt][:, :, 0:64],
                        pv[:].rearrange("p (h d) -> p h d", h=NH),
                        bv_sb[:].rearrange("p (h d) -> p h d", h=NH))

                # ---- attention + output projection for q-chunk c = s ----
                c = s
                njt = min(4 * c + 4, NKT)     # causal: k-tiles 0..4c+3
                nfull = 4 * c                 # k-tiles fully below the diagonal
                attn_n = {}

                def emit_scores(t, par):
                    # S^T and P^T for head (t,par): full k-tile pairs, then
                    # the 4 diagonal k-tiles with causal column trimming
                    av_full = []
                    av_diag = []
                    for g in range(nfull // 2):
                        st = stp.tile([P, 2 * QC], F32, tag="st")
                        for half in range(2):
                            j = 2 * g + half
                            nc.tensor.matmul(
                                st[:, half * QC:(half + 1) * QC],
                                kT_sb[t][par * DH:(par + 1) * DH,
                                         j * KT:(j + 1) * KT],
                                qT_t[(t, c)][par * DH:(par + 1) * DH, :],
                                start=True, stop=True)
                        pt = ptp.tile([P, 2 * QC], BF16, tag="pt")
                        nc.scalar.activation(
                            out=pt, in_=st,
                            func=mybir.ActivationFunctionType.Exp,
                            scale=float(scale))
                        av_full.append((2 * g, pt[:, 0:QC]))
                        av_full.append((2 * g + 1, pt[:, QC:2 * QC]))
                    for g in range(2):
                        st = stp.tile([P, 2 * QC], F32, tag="st")
                        for half in range(2):
                            m = 2 * g + half
                            j = nfull + m
                            lo = P * m
                            nc.tensor.matmul(
                                st[:, half * QC + lo:half * QC + QC],
                                kT_sb[t][par * DH:(par + 1) * DH,
                                         j * KT:(j + 1) * KT],
                                qT_t[(t, c)][par * DH:(par + 1) * DH, lo:QC],
                                start=True, stop=True)
                            pd = ptd[m][par]
                            nc.scalar.activation(
                                out=pd[:, lo:QC],
                                in_=st[:, half * QC + lo:half * QC + QC],
                                func=mybir.ActivationFunctionType.Exp,
                                scale=float(scale))
                            nc.gpsimd.tensor_mul(
                                pd[:, lo:lo + P], pd[:, lo:lo + P], tri_sb)
                            av_diag.append((j, pd, lo))
                    return (t, par, av_full, av_diag)

                def emit_av_norm(ctx):
                    t, par, av_full, av_diag = ctx
                    h = 2 * t + par
                    an_t = attn_n[t]
                    # AV with fused ones column -> rows 0..63 attn, row
                    # 64 = Z.  Diagonal tiles stream first (only their
                    # valid column suffix when a full tile follows to
                    # close the accumulation group); full tiles last.
                    acc = accp.tile([P, QC], F32, tag="acc")
                    for idx, (j, pd, lo) in enumerate(av_diag):
                        if nfull == 0:
                            lo = 0      # no full tile after: keep the
                                        # group full-width (prefix is 0)
                        nc.tensor.matmul(
                            acc[0:65, lo:QC],
                            v_sb[j][:, h, :],
                            pd[:, lo:QC],
                            start=(idx == 0),
                            stop=(nfull == 0 and idx == 3))
                    for idx, (j, rhs) in enumerate(av_full):
                        nc.tensor.matmul(
                            acc[0:65, :],
                            v_sb[j][:, h, :],
                            rhs,
                            start=False, stop=(idx == nfull - 1))
                    # normalization: broadcast Z over 64 rows via a DMA
                    # round-trip through DRAM (lane shift), then a fast
                    # approximate reciprocal on partitions 0..63
                    zraw = zrp.tile([P, QC], F32, tag="zraw")
                    bzs = bzsb.tile([P, QC], F32, tag="bzs")
                    zrow = zdp.tile([1, QC], F32, tag="zd", name="zrow")
                    # DMA cannot read PSUM: bounce Z through SBUF
                    nc.vector.tensor_copy(zraw[64:65, :], acc[64:65, :])
                    nc.sync.dma_start(out=zrow, in_=zraw[64:65, :])
                    nc.sync.dma_start(out=zraw[0:DH, :].unsqueeze(1),
                                      in_=zrow.partition_broadcast(DH))
                    nc.vector.reciprocal_approx_fast(
                        out=bzs[0:DH, :], in_=zraw[0:DH, :])
                    if par == 0:
                        nc.vector.tensor_mul(
                            an_t[0:DH, :], acc[0:DH, :], bzs[0:DH, :])
                    else:
                        an_o = anodd.tile([DH, QC], BF16, tag="anodd")
                        nc.vector.tensor_mul(
                            an_o, acc[0:DH, :], bzs[0:DH, :])
                        # shift to partitions 64..127 (DMA can cross lanes)
                        nc.sync.dma_start(out=an_t[DH:P, :], in_=an_o)

                def emit_oproj(oc):
                    # output projection for chunk oc + pipelined bf16
                    # pair-ReduceScatter + output slice write
                    an_c = attn_by_chunk.pop(oc)
                    ot = otp.tile([P, D // P, QC], BF16, tag="ot")
                    for o in range(D // P):
                        po = scratch.tile([P, QC], F32, tag="pacc")
                        for t in range(HC // P):
                            nc.tensor.matmul(
                                po,
                                wo_sb[t][:, o * P:(o + 1) * P],
                                an_c[t],
                                start=(t == 0), stop=(t == HC // P - 1))
                        nc.vector.tensor_scalar_add(
                            ot[:, o, :], po, bo_sb[:, o:o + 1])
                    # one batched DMA for all 8 o-tiles
                    nc.sync.dma_start(
                        out=partT[oc][:, :].rearrange("(o p) q -> p o q", o=D // P),
                        in_=ot[:, :, :])
                    nc.gpsimd.collective_compute(
                        "ReduceScatter", mybir.AluOpType.add,
                        replica_groups=[[0, 1], [2, 3], [4, 5], [6, 7]],
                        ins=[partT[oc].opt()], outs=[rs_out[oc].opt()],
                    )
                    nc.sync.dma_start(
                        out=outTh[:, oc * QC:(oc + 1) * QC], in_=rs_out[oc][:, :])

                # software-pipeline the heads: emit S^T/exp of head i+1
                # before the AV of head i so the in-order PE queue always
                # has independent matmuls while the Scalar engine runs exp
                pending = None
                for t in range(HC // P):
                    attn_n[t] = anp.tile([P, QC], BF16, tag="an", name="an_t")
                    for par in range(2):
                        ctx = emit_scores(t, par)
                        if pending is not None:
                            emit_av_norm(pending)
                        pending = ctx
                attn_by_chunk[c] = attn_n
                # previous chunk's O-projection fills the PE while the last
                # head's exp runs; then flush the pending AV
                if c - 1 >= 0:
                    emit_oproj(c - 1)
                emit_av_norm(pending)
                if c == NQC - 1:
                    emit_oproj(c)

    nc.compile()
    return nc


def _make_in_maps(x, Wq, bq, Wk, bk, Wv, bv, Wo, bo, mask):
    ref = np.tril(np.ones((L, L), dtype=np.int32))[None, None]
    assert np.array_equal(np.asarray(mask), ref), "mask must be causal"

    # triangle pattern for the diagonal [128,128] block: key p attends q f
    # iff p <= f
    tri = (np.arange(P)[:, None] <= np.arange(P)[None, :]).astype(np.float32)

    in_maps = []
    for c in range(N_CORES):
        b, g = c // 2, c % 2
        cols = slice(HC * g, HC * g + HC)
        in_maps.append({
            "xT": np.ascontiguousarray(np.asarray(x[b]).T).astype(ml_dtypes.bfloat16),
            "wq": np.ascontiguousarray(np.asarray(Wq)[:, cols]).astype(ml_dtypes.bfloat16),
            "wk": np.ascontiguousarray(np.asarray(Wk)[:, cols]).astype(ml_dtypes.bfloat16),
            "wv": np.ascontiguousarray(np.asarray(Wv)[:, cols]).astype(ml_dtypes.bfloat16),
            "wo": np.ascontiguousarray(np.asarray(Wo)[cols, :]).astype(ml_dtypes.bfloat16),
            "bq": np.ascontiguousarray(np.asarray(bq)[cols].reshape(HC // P, P).T),
            "bk": np.ascontiguousarray(np.asarray(bk)[cols].reshape(HC // P, P).T),
            "bv": np.ascontiguousarray(
                np.broadcast_to(np.asarray(bv)[cols], (P, HC))),
            "bo": np.ascontiguousarray(
                (np.asarray(bo) / 2.0).reshape(D // P, P).T.astype(np.float32)),
            "tri": tri.astype(ml_dtypes.bfloat16),
        })
    return in_maps


def kernel(x, Wq, bq, Wk, bk, Wv, bv, Wo, bo, mask):
    global _NC, LAST_EXEC_NS
    if _NC is None:
        _NC = build_nc()
    in_maps = _make_in_maps(x, Wq, bq, Wk, bk, Wv, bv, Wo, bo, mask)
    r = run_bass_kernel_spmd(
        _NC, in_maps, core_ids=list(range(N_CORES)), trace=TRACE)
    LAST_EXEC_NS = r.exec_time_ns
    out = np.empty((B, L, D), dtype=np.float32)
    for b in range(B):
        outT = np.concatenate(
            [r.results[2 * b]["outTh"].astype(np.float32),
             r.results[2 * b + 1]["outTh"].astype(np.float32)], axis=0)
        out[b] = outT.T
    return out


# revision 33
# speedup vs baseline: 1.5932x; 1.0194x over previous
"""Multi-head causal attention on 8 Trainium2 NeuronCores.

Sharding: core c handles batch b = c // 2 and head-group g = c % 2
(8 of 16 heads, i.e. 512 of 1024 projection columns).  QKV projections,
attention and the output projection partial run per-core; the two cores
of a batch pair-ReduceScatter their partial outputs (pipelined per
512-seq chunk so the collective overlaps compute).

Everything on-device is computed in a transposed layout (seq on the
free dim) so no PE transposes are needed anywhere:
  xT [D, L] (host-pre-transposed, bf16) -> qT/kT [512, L] bf16
  -> S^T [keys, q] -> P^T = exp(S^T) (bf16) -> attn^T = (v|ones)^T @ P^T
  -> out^T = Wo^T @ attn_norm^T.  Host un-transposes the result.

v2 changes vs baseline:
  - all matmul operands bf16 (was f32r) -> cheaper LDWEIGHTS, less SBUF BW
  - causal diagonal tiles: S^T/exp computed only for the valid column
    suffix; the [128,128] diagonal block is triangle-masked with one DVE
    mul; dedicated prefix-zeroed pt tiles replace full-width mask muls
  - softmax 1/Z via reciprocal_approx_fast (was 3.4us/op DVE reciprocal)
  - output-projection bias-add on DVE (tensor_scalar_add), frees Scalar
  - partial outputs in bf16, pair-ReduceScatter per chunk *inside* the
    tile context -> collective overlaps compute instead of a 134us tail
"""

import sys, types

sys.path.insert(0, "/opt/trn_rl_repo")

# antenv.axon_hooks is missing in this image; inject it so trace=True can
# reach the NTFF profiling hook (used by test.py, off by default).
if "antenv.axon_hooks" not in sys.modules:
    _hook_mod = types.ModuleType("antenv.axon_hooks")
    _hook_mod._hook = None
    def _set_hook(h):
        _hook_mod._hook = h
    def _get_hook():
        return _hook_mod._hook
    _hook_mod.set_axon_ntff_profile_hook = _set_hook
    _hook_mod.get_axon_ntff_profile_hook = _get_hook
    sys.modules["antenv.axon_hooks"] = _hook_mod
    try:
        import antenv
        antenv.axon_hooks = _hook_mod
        from trn_agent_boot.trn_boot import _ntff_profile_via_ctypes
        _set_hook(_ntff_profile_via_ctypes("/opt/axon/libaxon_pjrt.so"))
    except Exception:
        pass

import numpy as np
import ml_dtypes
import concourse.bass as bass
import concourse.mybir as mybir
import concourse.tile as tile
from concourse import bacc
from concourse.bass_utils import run_bass_kernel_spmd

B, L, D, H = 4, 2048, 1024, 16
DH = 64
N_CORES = 8
NH = 8          # heads per core
HC = NH * DH    # 512 projection cols per core
QC = 512        # q-chunk
KT = 128        # k-tile
P = 128

F32 = mybir.dt.float32
BF16 = mybir.dt.bfloat16

TRACE = False
LAST_EXEC_NS = None
_NC = None


def build_nc(seq_len=L):
    Ls = seq_len
    NQC = Ls // QC
    NKT = Ls // KT
    NDS = D // P       # 8 contraction tiles for projections
    nc = bacc.Bacc()

    # xT is chunk-major: row s*D + d holds x^T[d, s*QC:(s+1)*QC], so each
    # [128, QC] tile DMA is one contiguous 128 KB read
    xT = nc.declare_dram_parameter("xT", [(Ls // QC) * D, QC], BF16, isOutput=False)
    wq = nc.declare_dram_parameter("wq", [D, HC], BF16, isOutput=False)
    wk = nc.declare_dram_parameter("wk", [D, HC], BF16, isOutput=False)
    wv = nc.declare_dram_parameter("wv", [D, HC], BF16, isOutput=False)
    wo = nc.declare_dram_parameter("wo", [HC, D], BF16, isOutput=False)
    bq = nc.declare_dram_parameter("bq", [P, HC // P], F32, isOutput=False)
    bk = nc.declare_dram_parameter("bk", [P, HC // P], F32, isOutput=False)
    bv = nc.declare_dram_parameter("bv", [P, HC], F32, isOutput=False)
    bo = nc.declare_dram_parameter("bo", [P, D // P], F32, isOutput=False)
    tri = nc.declare_dram_parameter("tri", [P, P], BF16, isOutput=False)
    # outTh is chunk-major as well: rows [c*(D//2), (c+1)*(D//2)) hold chunk c
    outTh = nc.declare_dram_parameter(
        "outTh", [(Ls // QC) * (D // 2), QC], BF16, isOutput=True)

    scale = 1.0 / np.sqrt(np.float32(DH))

    from contextlib import ExitStack
    with nc.allow_low_precision(reason="bf16 matmuls by design; tol 2e-2"), \
         tile.TileContext(nc) as tc, ExitStack() as ctx:
        consts = ctx.enter_context(tc.tile_pool(name="consts", bufs=1))
        wpool = ctx.enter_context(tc.tile_pool(name="wpool", bufs=1))
        kvres = ctx.enter_context(tc.tile_pool(name="kvres", bufs=1))
        xtp = ctx.enter_context(tc.tile_pool(name="xtp", bufs=16))
        qtp = ctx.enter_context(tc.tile_pool(name="qtp", bufs=8))
        ptp = ctx.enter_context(tc.tile_pool(name="ptp", bufs=8))
        anp = ctx.enter_context(tc.tile_pool(name="anp", bufs=8))
        otp = ctx.enter_context(tc.tile_pool(name="otp", bufs=2))
        zrp = ctx.enter_context(tc.tile_pool(name="zrp", bufs=2))
        bzsb = ctx.enter_context(tc.tile_pool(name="bzsb", bufs=2))
        anodd = ctx.enter_context(tc.tile_pool(name="anodd", bufs=2))
        zdp = ctx.enter_context(tc.tile_pool(name="zdp", bufs=4, space="DRAM"))
        dramp = ctx.enter_context(tc.tile_pool(name="dramp", bufs=1, space="DRAM"))
        scratch = ctx.enter_context(tc.tile_pool(name="scratch", bufs=2, space="PSUM"))
        stp = ctx.enter_context(tc.tile_pool(name="stp", bufs=2, space="PSUM"))
        accp = ctx.enter_context(tc.tile_pool(name="accp", bufs=2, space="PSUM"))

        if True:
            # per-chunk bounce buffers for the pair-ReduceScatter (pool
            # tiles so the Tile framework tracks the DMA -> CC -> DMA deps);
            # the last chunk is split into two column halves so its final
            # ReduceScatter is half as long in the drain tail
            partT = [dramp.tile([D, QC], BF16, tag=f"partT{c}", name=f"partT{c}")
                     for c in range(NQC)]
            partTh = [dramp.tile([D, QC // 2], BF16, tag=f"partTh{i}", name=f"partTh{i}")
                      for i in range(2)]
            rs_out = [dramp.tile([D // 2, QC], BF16, tag=f"rs{c}", name=f"rs{c}")
                      for c in range(NQC)]
            rs_outh = [dramp.tile([D // 2, QC // 2], BF16, tag=f"rsh{i}", name=f"rsh{i}")
                       for i in range(2)]

            # ---- constants ----
            bq_sb = consts.tile([P, HC // P], F32, tag="bq")
            bk_sb = consts.tile([P, HC // P], F32, tag="bk")
            bv_sb = consts.tile([P, HC], F32, tag="bv")
            bo_sb = consts.tile([P, D // P], F32, tag="bo")
            tri_sb = consts.tile([P, P], BF16, tag="tri")
            nc.sync.dma_start(out=bq_sb, in_=bq[:, :])
            nc.sync.dma_start(out=bk_sb, in_=bk[:, :])
            nc.sync.dma_start(out=bv_sb, in_=bv[:, :])
            nc.sync.dma_start(out=bo_sb, in_=bo[:, :])
            nc.sync.dma_start(out=tri_sb, in_=tri[:, :])

            # ---- weights resident ----
            wq_sb = [wpool.tile([P, HC], BF16, tag=f"wq{ds}", name=f"wq{ds}") for ds in range(NDS)]
            wk_sb = [wpool.tile([P, HC], BF16, tag=f"wk{ds}", name=f"wk{ds}") for ds in range(NDS)]
            wv_sb = [wpool.tile([P, HC], BF16, tag=f"wv{ds}", name=f"wv{ds}") for ds in range(NDS)]
            wo_sb = [wpool.tile([P, D], BF16, tag=f"wo{t}", name=f"wo{t}") for t in range(HC // P)]
            # wq + chunk-0 xT interleaved on the sync queue (both needed for
            # the first Q matmuls); wk/wv/wo on other engines' DMA queues so
            # the rings run in parallel and the first matmul isn't stuck
            # behind 4 MB of weight traffic
            xT_t = {}

            def fetch_x(s):
                for ds in range(NDS):
                    xt = xtp.tile([P, QC], BF16, tag="xT")
                    nc.sync.dma_start(
                        out=xt, in_=xT[s * D + ds * P:s * D + (ds + 1) * P, :])
                    xT_t[(ds, s)] = xt

            for ds in range(NDS):
                nc.sync.dma_start(out=wq_sb[ds], in_=wq[ds * P:(ds + 1) * P, :])
                xt = xtp.tile([P, QC], BF16, tag="xT")
                nc.sync.dma_start(out=xt, in_=xT[ds * P:(ds + 1) * P, :])
                xT_t[(ds, 0)] = xt
            for ds in range(NDS):
                nc.scalar.dma_start(out=wk_sb[ds], in_=wk[ds * P:(ds + 1) * P, :])
                nc.gpsimd.dma_start(out=wv_sb[ds], in_=wv[ds * P:(ds + 1) * P, :])
            for t in range(HC // P):
                nc.gpsimd.dma_start(out=wo_sb[t], in_=wo[t * P:(t + 1) * P, :])

            # ---- resident kT and v ----
            kT_sb = [kvres.tile([P, Ls], BF16, tag=f"kT{t}", name=f"kT{t}") for t in range(HC // P)]
            # v: per key-tile [128, NH, 65] bf16; cols 0..63 = v, col 64 = ones
            # (the ones column makes the AV matmul emit softmax Z in row 64)
            v_sb = [kvres.tile([P, NH, 65], BF16, tag=f"v{kt}", name=f"v{kt}") for kt in range(NKT)]
            for kt in range(NKT):
                # only the ones-column needs initialization; cols 0..63 are
                # fully overwritten by the v-projection bias add
                nc.vector.memset(v_sb[kt][:, :, 64:65], 1.0)

            # ---- dedicated diagonal-P^T tiles: for diag k-tile m (= j-4c),
            # exp writes cols [128m, 512); the prefix [0, 128m) must read as
            # zero in the AV matmul, so it is zeroed ONCE here and never
            # written again. 2 buffers ping-pong across heads. ----
            ptd = [[kvres.tile([P, QC], BF16, tag=f"ptd{m}_{b_}", name=f"ptd{m}_{b_}")
                    for b_ in range(2)] for m in range(4)]
            for m in range(1, 4):
                for b_ in range(2):
                    nc.vector.memset(ptd[m][b_][:, 0:P * m], 0.0)

            # ---- per-chunk: projections for chunk s, then attention and
            # output projection for q-chunk c=s (causal => only needs k/v
            # from chunks <= s) ----
            qT_t = {}
            attn_by_chunk = {}
            for s in range(NQC):
                assert (0, s) in xT_t  # prefetched (chunk 0 with the weights)

                for t in range(HC // P):
                    # qT tile [128 outcol, QC seq]
                    pq = scratch.tile([P, QC], F32, tag="pacc")
                    for ds in range(NDS):
                        nc.tensor.matmul(
                            pq,
                            wq_sb[ds][:, t * P:(t + 1) * P],
                            xT_t[(ds, s)],
                            start=(ds == 0), stop=(ds == NDS - 1))
                    qt = qtp.tile([P, QC], BF16, tag="qT")
                    nc.vector.tensor_scalar_add(qt, pq, bq_sb[:, t:t + 1])
                    qT_t[(t, s)] = qt

                    pk = scratch.tile([P, QC], F32, tag="pacc")
                    for ds in range(NDS):
                        nc.tensor.matmul(
                            pk,
                            wk_sb[ds][:, t * P:(t + 1) * P],
                            xT_t[(ds, s)],
                            start=(ds == 0), stop=(ds == NDS - 1))
                    nc.vector.tensor_scalar_add(
                        kT_sb[t][:, s * QC:(s + 1) * QC], pk, bk_sb[:, t:t + 1])

                # v for the 4 key-tiles of this seq chunk
                for sub in range(QC // P):
                    kt = s * (QC // P) + sub
                    pv = scratch.tile([P, HC], F32, tag="pacc")
                    for ds in range(NDS):
                        nc.tensor.matmul(
                            pv,
                            xT_t[(ds, s)][:, sub * P:(sub + 1) * P],
                            wv_sb[ds],
                            start=(ds == 0), stop=(ds == NDS - 1))
                    # gpsimd cannot read PSUM; this stays on DVE
                    nc.vector.tensor_add(
                        v_sb[kt][:, :, 0:64],
                        pv[:].rearrange("p (h d) -> p h d", h=NH),
                        bv_sb[:].rearrange("p (h d) -> p h d", h=NH))

                # prefetch next chunk's x tiles so the chunk boundary isn't
                # gated on their DMA
                if s + 1 < NQC:
                    fetch_x(s + 1)

                # ---- attention + output projection for q-chunk c = s ----
                c = s
                njt = min(4 * c + 4, NKT)     # causal: k-tiles 0..4c+3
                nfull = 4 * c                 # k-tiles fully below the diagonal
                attn_n = {}

                def emit_scores(t, par):
                    # S^T and P^T for head (t,par): full k-tile pairs, then
                    # the 4 diagonal k-tiles with causal column trimming
                    av_full = []
                    av_diag = []
                    for g in range(nfull // 2):
                        st = stp.tile([P, 2 * QC], F32, tag="st")
                        for half in range(2):
                            j = 2 * g + half
                            nc.tensor.matmul(
                                st[:, half * QC:(half + 1) * QC],
                                kT_sb[t][par * DH:(par + 1) * DH,
                                         j * KT:(j + 1) * KT],
                                qT_t[(t, c)][par * DH:(par + 1) * DH, :],
                                start=True, stop=True)
                        pt = ptp.tile([P, 2 * QC], BF16, tag="pt")
                        nc.scalar.activation(
                            out=pt, in_=st,
                            func=mybir.ActivationFunctionType.Exp,
                            scale=float(scale))
                        av_full.append((2 * g, pt[:, 0:QC]))
                        av_full.append((2 * g + 1, pt[:, QC:2 * QC]))
                    for g in range(2):
                        st = stp.tile([P, 2 * QC], F32, tag="st")
                        for half in range(2):
                            m = 2 * g + half
                            j = nfull + m
                            lo = P * m
                            nc.tensor.matmul(
                                st[:, half * QC + lo:half * QC + QC],
                                kT_sb[t][par * DH:(par + 1) * DH,
                                         j * KT:(j + 1) * KT],
                                qT_t[(t, c)][par * DH:(par + 1) * DH, lo:QC],
                                start=True, stop=True)
                            pd = ptd[m][par]
                            nc.scalar.activation(
                                out=pd[:, lo:QC],
                                in_=st[:, half * QC + lo:half * QC + QC],
                                func=mybir.ActivationFunctionType.Exp,
                                scale=float(scale))
                            nc.gpsimd.tensor_mul(
                                pd[:, lo:lo + P], pd[:, lo:lo + P], tri_sb)
                            av_diag.append((j, pd, lo))
                    return (t, par, av_full, av_diag)

                def emit_av_norm(ctx):
                    t, par, av_full, av_diag = ctx
                    h = 2 * t + par
                    an_t = attn_n[t]
                    # AV with fused ones column -> rows 0..63 attn, row
                    # 64 = Z.  Diagonal tiles stream first (only their
                    # valid column suffix when a full tile follows to
                    # close the accumulation group); full tiles last.
                    acc = accp.tile([P, QC], F32, tag="acc")
                    for idx, (j, pd, lo) in enumerate(av_diag):
                        if nfull == 0:
                            lo = 0      # no full tile after: keep the
                                        # group full-width (prefix is 0)
                        nc.tensor.matmul(
                            acc[0:65, lo:QC],
                            v_sb[j][:, h, :],
                            pd[:, lo:QC],
                            start=(idx == 0),
                            stop=(nfull == 0 and idx == 3))
                    for idx, (j, rhs) in enumerate(av_full):
                        nc.tensor.matmul(
                            acc[0:65, :],
                            v_sb[j][:, h, :],
                            rhs,
                            start=False, stop=(idx == nfull - 1))
                    # normalization: broadcast Z over 64 rows via a DMA
                    # round-trip through DRAM (lane shift), then a fast
                    # approximate reciprocal on partitions 0..63
                    zraw = zrp.tile([P, QC], F32, tag="zraw")
                    bzs = bzsb.tile([P, QC], F32, tag="bzs")
                    zrow = zdp.tile([1, QC], F32, tag="zd", name="zrow")
                    # DMA cannot read PSUM: bounce Z through SBUF
                    nc.vector.tensor_copy(zraw[64:65, :], acc[64:65, :])
                    nc.sync.dma_start(out=zrow, in_=zraw[64:65, :])
                    nc.sync.dma_start(out=zraw[0:DH, :].unsqueeze(1),
                                      in_=zrow.partition_broadcast(DH))
                    nc.vector.reciprocal_approx_fast(
                        out=bzs[0:DH, :], in_=zraw[0:DH, :])
                    if par == 0:
                        nc.vector.tensor_mul(
                            an_t[0:DH, :], acc[0:DH, :], bzs[0:DH, :])
                    else:
                        an_o = anodd.tile([DH, QC], BF16, tag="anodd")
                        nc.vector.tensor_mul(
                            an_o, acc[0:DH, :], bzs[0:DH, :])
                        # shift to partitions 64..127 (DMA can cross lanes)
                        nc.sync.dma_start(out=an_t[DH:P, :], in_=an_o)

                def emit_oproj(oc):
                    # output projection for chunk oc + pipelined bf16
                    # pair-ReduceScatter + output slice write.  The last
                    # chunk runs in two column halves so the second (tail)
                    # ReduceScatter is half-sized.
                    an_c = attn_by_chunk.pop(oc)
                    if oc < NQC - 1:
                        pieces = [(partT[oc], rs_out[oc], 0, QC)]
                    else:
                        pieces = [(partTh[0], rs_outh[0], 0, QC // 2),
                                  (partTh[1], rs_outh[1], QC // 2, QC)]
                    for pT, rso, q0, q1 in pieces:
                        qn = q1 - q0
                        ot = otp.tile([P, D // P, QC], BF16, tag="ot")
                        for o in range(D // P):
                            po = scratch.tile([P, QC], F32, tag="pacc")
                            for t in range(HC // P):
                                nc.tensor.matmul(
                                    po[:, 0:qn],
                                    wo_sb[t][:, o * P:(o + 1) * P],
                                    an_c[t][:, q0:q1],
                                    start=(t == 0), stop=(t == HC // P - 1))
                            nc.vector.tensor_scalar_add(
                                ot[:, o, 0:qn], po[:, 0:qn], bo_sb[:, o:o + 1])
                        # one batched DMA for all 8 o-tiles
                        nc.sync.dma_start(
                            out=pT[:, :].rearrange("(o p) q -> p o q", o=D // P),
                            in_=ot[:, :, 0:qn])
                        nc.gpsimd.collective_compute(
                            "ReduceScatter", mybir.AluOpType.add,
                            replica_groups=[[0, 1], [2, 3], [4, 5], [6, 7]],
                            ins=[pT.opt()], outs=[rso.opt()],
                        )
                        nc.sync.dma_start(
                            out=outTh[oc * (D // 2):(oc + 1) * (D // 2), q0:q1],
                            in_=rso[:, :])

                # software-pipeline the heads: emit S^T/exp of head i+1
                # before the AV of head i so the in-order PE queue always
                # has independent matmuls while the Scalar engine runs exp
                pending = None
                for t in range(HC // P):
                    attn_n[t] = anp.tile([P, QC], BF16, tag="an", name="an_t")
                    # par=1 first: its lane-shift DMA then overlaps par=0's
                    # compute, and the an tile's last writer is par=0's
                    # direct DVE mul (no DMA on the O-projection's
                    # critical path)
                    for par in (1, 0):
                        ctx = emit_scores(t, par)
                        if pending is not None:
                            emit_av_norm(pending)
                        pending = ctx
                attn_by_chunk[c] = attn_n
                # previous chunk's O-projection fills the PE while the last
                # head's exp runs; then flush the pending AV
                if c - 1 >= 0:
                    emit_oproj(c - 1)
                emit_av_norm(pending)
                if c == NQC - 1:
                    emit_oproj(c)

    nc.compile()
    return nc


def _make_in_maps(x, Wq, bq, Wk, bk, Wv, bv, Wo, bo, mask):
    ref = np.tril(np.ones((L, L), dtype=np.int32))[None, None]
    assert np.array_equal(np.asarray(mask), ref), "mask must be causal"

    # triangle pattern for the diagonal [128,128] block: key p attends q f
    # iff p <= f
    tri = (np.arange(P)[:, None] <= np.arange(P)[None, :]).astype(np.float32)

    in_maps = []
    for c in range(N_CORES):
        b, g = c // 2, c % 2
        cols = slice(HC * g, HC * g + HC)
        # chunk-major x^T: [NQC*D, QC], row s*D + d = x^T[d, s*QC:(s+1)*QC]
        xTb = np.asarray(x[b]).T.reshape(D, L // QC, QC).transpose(1, 0, 2)
        in_maps.append({
            "xT": np.ascontiguousarray(xTb.reshape(-1, QC)).astype(ml_dtypes.bfloat16),
            "wq": np.ascontiguousarray(np.asarray(Wq)[:, cols]).astype(ml_dtypes.bfloat16),
            "wk": np.ascontiguousarray(np.asarray(Wk)[:, cols]).astype(ml_dtypes.bfloat16),
            "wv": np.ascontiguousarray(np.asarray(Wv)[:, cols]).astype(ml_dtypes.bfloat16),
            "wo": np.ascontiguousarray(np.asarray(Wo)[cols, :]).astype(ml_dtypes.bfloat16),
            "bq": np.ascontiguousarray(np.asarray(bq)[cols].reshape(HC // P, P).T),
            "bk": np.ascontiguousarray(np.asarray(bk)[cols].reshape(HC // P, P).T),
            "bv": np.ascontiguousarray(
                np.broadcast_to(np.asarray(bv)[cols], (P, HC))),
            "bo": np.ascontiguousarray(
                (np.asarray(bo) / 2.0).reshape(D // P, P).T.astype(np.float32)),
            "tri": tri.astype(ml_dtypes.bfloat16),
        })
    return in_maps


def kernel(x, Wq, bq, Wk, bk, Wv, bv, Wo, bo, mask):
    global _NC, LAST_EXEC_NS
    if _NC is None:
        _NC = build_nc()
    in_maps = _make_in_maps(x, Wq, bq, Wk, bk, Wv, bv, Wo, bo, mask)
    r = run_bass_kernel_spmd(
        _NC, in_maps, core_ids=list(range(N_CORES)), trace=TRACE)
    LAST_EXEC_NS = r.exec_time_ns
    out = np.empty((B, L, D), dtype=np.float32)
    for b in range(B):
        # outTh is chunk-major [NQC*(D//2), QC] -> [D//2, L]
        halves = []
        for cc in (2 * b, 2 * b + 1):
            oT = r.results[cc]["outTh"].astype(np.float32)
            halves.append(
                oT.reshape(L // QC, D // 2, QC).transpose(1, 0, 2).reshape(D // 2, L))
        out[b] = np.concatenate(halves, axis=0).T
    return out


# revision 39
# speedup vs baseline: 1.6389x; 1.0287x over previous
"""Multi-head causal attention on 8 Trainium2 NeuronCores.

Sharding: core c handles batch b = c // 2 and head-group g = c % 2
(8 of 16 heads, i.e. 512 of 1024 projection columns).  QKV projections,
attention and the output projection partial run per-core; the two cores
of a batch pair-ReduceScatter their partial outputs (pipelined per
512-seq chunk so the collective overlaps compute).

Everything on-device is computed in a transposed layout (seq on the
free dim) so no PE transposes are needed anywhere:
  xT [D, L] (host-pre-transposed, bf16) -> qT/kT [512, L] bf16
  -> S^T [keys, q] -> P^T = exp(S^T) (bf16) -> attn^T = (v|ones)^T @ P^T
  -> out^T = Wo^T @ attn_norm^T.  Host un-transposes the result.

v2 changes vs baseline:
  - all matmul operands bf16 (was f32r) -> cheaper LDWEIGHTS, less SBUF BW
  - causal diagonal tiles: S^T/exp computed only for the valid column
    suffix; the [128,128] diagonal block is triangle-masked with one DVE
    mul; dedicated prefix-zeroed pt tiles replace full-width mask muls
  - softmax 1/Z via reciprocal_approx_fast (was 3.4us/op DVE reciprocal)
  - output-projection bias-add on DVE (tensor_scalar_add), frees Scalar
  - partial outputs in bf16, pair-ReduceScatter per chunk *inside* the
    tile context -> collective overlaps compute instead of a 134us tail
"""

import sys, types

sys.path.insert(0, "/opt/trn_rl_repo")

# antenv.axon_hooks is missing in this image; inject it so trace=True can
# reach the NTFF profiling hook (used by test.py, off by default).
if "antenv.axon_hooks" not in sys.modules:
    _hook_mod = types.ModuleType("antenv.axon_hooks")
    _hook_mod._hook = None
    def _set_hook(h):
        _hook_mod._hook = h
    def _get_hook():
        return _hook_mod._hook
    _hook_mod.set_axon_ntff_profile_hook = _set_hook
    _hook_mod.get_axon_ntff_profile_hook = _get_hook
    sys.modules["antenv.axon_hooks"] = _hook_mod
    try:
        import antenv
        antenv.axon_hooks = _hook_mod
        from trn_agent_boot.trn_boot import _ntff_profile_via_ctypes
        _set_hook(_ntff_profile_via_ctypes("/opt/axon/libaxon_pjrt.so"))
    except Exception:
        pass

import numpy as np
import ml_dtypes
import concourse.bass as bass
import concourse.mybir as mybir
import concourse.tile as tile
from concourse import bacc
from concourse.bass_utils import run_bass_kernel_spmd

B, L, D, H = 4, 2048, 1024, 16
DH = 64
N_CORES = 8
NH = 8          # heads per core
HC = NH * DH    # 512 projection cols per core
QC = 512        # q-chunk
KT = 128        # k-tile
P = 128

F32 = mybir.dt.float32
BF16 = mybir.dt.bfloat16

TRACE = False
LAST_EXEC_NS = None
_NC = None


def build_nc(seq_len=L):
    Ls = seq_len
    NQC = Ls // QC
    NKT = Ls // KT
    NDS = D // P       # 8 contraction tiles for projections
    nc = bacc.Bacc()

    # xT is chunk-major: row s*D + d holds x^T[d, s*QC:(s+1)*QC], so each
    # [128, QC] tile DMA is one contiguous 128 KB read
    xT = nc.declare_dram_parameter("xT", [(Ls // QC) * D, QC], BF16, isOutput=False)
    wq = nc.declare_dram_parameter("wq", [D, HC], BF16, isOutput=False)
    wk = nc.declare_dram_parameter("wk", [D, HC], BF16, isOutput=False)
    wv = nc.declare_dram_parameter("wv", [D, HC], BF16, isOutput=False)
    wo = nc.declare_dram_parameter("wo", [HC, D], BF16, isOutput=False)
    bq = nc.declare_dram_parameter("bq", [P, HC // P], F32, isOutput=False)
    bk = nc.declare_dram_parameter("bk", [P, HC // P], F32, isOutput=False)
    bv = nc.declare_dram_parameter("bv", [1, HC], F32, isOutput=False)
    bo = nc.declare_dram_parameter("bo", [P, D // P], F32, isOutput=False)
    tri = nc.declare_dram_parameter("tri", [P, P], BF16, isOutput=False)
    # outTh is chunk-major as well: rows [c*(D//2), (c+1)*(D//2)) hold chunk c
    outTh = nc.declare_dram_parameter(
        "outTh", [(Ls // QC) * (D // 2), QC], BF16, isOutput=True)

    scale = 1.0 / np.sqrt(np.float32(DH))

    from contextlib import ExitStack
    with nc.allow_low_precision(reason="bf16 matmuls by design; tol 2e-2"), \
         tile.TileContext(nc) as tc, ExitStack() as ctx:
        consts = ctx.enter_context(tc.tile_pool(name="consts", bufs=1))
        wpool = ctx.enter_context(tc.tile_pool(name="wpool", bufs=1))
        kvres = ctx.enter_context(tc.tile_pool(name="kvres", bufs=1))
        xtp = ctx.enter_context(tc.tile_pool(name="xtp", bufs=16))
        qtp = ctx.enter_context(tc.tile_pool(name="qtp", bufs=8))
        ptp = ctx.enter_context(tc.tile_pool(name="ptp", bufs=8))
        anp = ctx.enter_context(tc.tile_pool(name="anp", bufs=8))
        otp = ctx.enter_context(tc.tile_pool(name="otp", bufs=2))
        zrp = ctx.enter_context(tc.tile_pool(name="zrp", bufs=2))
        bzsb = ctx.enter_context(tc.tile_pool(name="bzsb", bufs=2))
        anodd = ctx.enter_context(tc.tile_pool(name="anodd", bufs=2))
        zdp = ctx.enter_context(tc.tile_pool(name="zdp", bufs=4, space="DRAM"))
        dramp = ctx.enter_context(tc.tile_pool(name="dramp", bufs=1, space="DRAM"))
        scratch = ctx.enter_context(tc.tile_pool(name="scratch", bufs=2, space="PSUM"))
        stp = ctx.enter_context(tc.tile_pool(name="stp", bufs=2, space="PSUM"))
        accp = ctx.enter_context(tc.tile_pool(name="accp", bufs=2, space="PSUM"))

        if True:
            # per-chunk bounce buffers for the pair-ReduceScatter (pool
            # tiles so the Tile framework tracks the DMA -> CC -> DMA deps);
            # the last chunk is split into two column halves so its final
            # ReduceScatter is half as long in the drain tail
            partT = [dramp.tile([D, QC], BF16, tag=f"partT{c}", name=f"partT{c}")
                     for c in range(NQC)]
            partTh = [dramp.tile([D, QC // 2], BF16, tag=f"partTh{i}", name=f"partTh{i}")
                      for i in range(2)]
            rs_out = [dramp.tile([D // 2, QC], BF16, tag=f"rs{c}", name=f"rs{c}")
                      for c in range(NQC)]
            rs_outh = [dramp.tile([D // 2, QC // 2], BF16, tag=f"rsh{i}", name=f"rsh{i}")
                       for i in range(2)]

            # ---- constants ----
            bq_sb = consts.tile([P, HC // P], F32, tag="bq")
            bk_sb = consts.tile([P, HC // P], F32, tag="bk")
            bv_sb = consts.tile([P, HC], F32, tag="bv")
            bo_sb = consts.tile([P, D // P], F32, tag="bo")
            tri_sb = consts.tile([P, P], BF16, tag="tri")
            nc.sync.dma_start(out=bq_sb, in_=bq[:, :])
            nc.sync.dma_start(out=bk_sb, in_=bk[:, :])
            nc.sync.dma_start(out=bv_sb.unsqueeze(1),
                              in_=bv[0:1, :].partition_broadcast(P))
            nc.sync.dma_start(out=bo_sb, in_=bo[:, :])
            nc.sync.dma_start(out=tri_sb, in_=tri[:, :])

            # ---- weights resident ----
            wq_sb = [wpool.tile([P, HC], BF16, tag=f"wq{ds}", name=f"wq{ds}") for ds in range(NDS)]
            wk_sb = [wpool.tile([P, HC], BF16, tag=f"wk{ds}", name=f"wk{ds}") for ds in range(NDS)]
            wv_sb = [wpool.tile([P, HC], BF16, tag=f"wv{ds}", name=f"wv{ds}") for ds in range(NDS)]
            wo_sb = [wpool.tile([P, D], BF16, tag=f"wo{t}", name=f"wo{t}") for t in range(HC // P)]
            # wq + chunk-0 xT interleaved on the sync queue (both needed for
            # the first Q matmuls); wk/wv/wo on other engines' DMA queues so
            # the rings run in parallel and the first matmul isn't stuck
            # behind 4 MB of weight traffic
            xT_t = {}

            def fetch_x(s):
                for ds in range(NDS):
                    xt = xtp.tile([P, QC], BF16, tag="xT")
                    nc.sync.dma_start(
                        out=xt, in_=xT[s * D + ds * P:s * D + (ds + 1) * P, :])
                    xT_t[(ds, s)] = xt

            for ds in range(NDS):
                nc.sync.dma_start(out=wq_sb[ds], in_=wq[ds * P:(ds + 1) * P, :])
                xt = xtp.tile([P, QC], BF16, tag="xT")
                nc.sync.dma_start(out=xt, in_=xT[ds * P:(ds + 1) * P, :])
                xT_t[(ds, 0)] = xt
            for ds in range(NDS):
                nc.scalar.dma_start(out=wk_sb[ds], in_=wk[ds * P:(ds + 1) * P, :])
                nc.gpsimd.dma_start(out=wv_sb[ds], in_=wv[ds * P:(ds + 1) * P, :])
            # wo is fetched later (emitted inside chunk-0's attention, behind
            # gpsimd compute) so its DMAs don't steal startup ring bandwidth

            # ---- resident kT and v ----
            kT_sb = [kvres.tile([P, Ls], BF16, tag=f"kT{t}", name=f"kT{t}") for t in range(HC // P)]
            # v: per key-tile [128, NH, 65] bf16; cols 0..63 = v, col 64 = ones
            # (the ones column makes the AV matmul emit softmax Z in row 64)
            v_sb = [kvres.tile([P, NH, 65], BF16, tag=f"v{kt}", name=f"v{kt}") for kt in range(NKT)]
            for kt in range(NKT):
                # only the ones-column needs initialization; cols 0..63 are
                # fully overwritten by the v-projection bias add
                nc.vector.memset(v_sb[kt][:, :, 64:65], 1.0)

            # ---- dedicated diagonal-P^T tiles: for diag k-tile m (= j-4c),
            # exp writes cols [128m, 512); the prefix [0, 128m) must read as
            # zero in the AV matmul, so it is zeroed ONCE here and never
            # written again. 2 buffers ping-pong across heads. ----
            ptd = [[kvres.tile([P, QC], BF16, tag=f"ptd{m}_{b_}", name=f"ptd{m}_{b_}")
                    for b_ in range(2)] for m in range(4)]
            for m in range(1, 4):
                for b_ in range(2):
                    nc.vector.memset(ptd[m][b_][:, 0:P * m], 0.0)

            # ---- per-chunk: projections for chunk s, then attention and
            # output projection for q-chunk c=s (causal => only needs k/v
            # from chunks <= s) ----
            qT_t = {}
            attn_by_chunk = {}
            for s in range(NQC):
                assert (0, s) in xT_t  # prefetched (chunk 0 with the weights)

                for t in range(HC // P):
                    # qT tile [128 outcol, QC seq]
                    pq = scratch.tile([P, QC], F32, tag="pacc")
                    for ds in range(NDS):
                        nc.tensor.matmul(
                            pq,
                            wq_sb[ds][:, t * P:(t + 1) * P],
                            xT_t[(ds, s)],
                            start=(ds == 0), stop=(ds == NDS - 1))
                    qt = qtp.tile([P, QC], BF16, tag="qT")
                    nc.vector.tensor_scalar_add(qt, pq, bq_sb[:, t:t + 1])
                    qT_t[(t, s)] = qt

                    pk = scratch.tile([P, QC], F32, tag="pacc")
                    for ds in range(NDS):
                        nc.tensor.matmul(
                            pk,
                            wk_sb[ds][:, t * P:(t + 1) * P],
                            xT_t[(ds, s)],
                            start=(ds == 0), stop=(ds == NDS - 1))
                    nc.vector.tensor_scalar_add(
                        kT_sb[t][:, s * QC:(s + 1) * QC], pk, bk_sb[:, t:t + 1])

                # v for the 4 key-tiles of this seq chunk
                for sub in range(QC // P):
                    kt = s * (QC // P) + sub
                    pv = scratch.tile([P, HC], F32, tag="pacc")
                    for ds in range(NDS):
                        nc.tensor.matmul(
                            pv,
                            xT_t[(ds, s)][:, sub * P:(sub + 1) * P],
                            wv_sb[ds],
                            start=(ds == 0), stop=(ds == NDS - 1))
                    # gpsimd cannot read PSUM; this stays on DVE
                    nc.vector.tensor_add(
                        v_sb[kt][:, :, 0:64],
                        pv[:].rearrange("p (h d) -> p h d", h=NH),
                        bv_sb[:].rearrange("p (h d) -> p h d", h=NH))

                # prefetch next chunk's x tiles so the chunk boundary isn't
                # gated on their DMA
                if s + 1 < NQC:
                    fetch_x(s + 1)

                # ---- attention + output projection for q-chunk c = s ----
                c = s
                njt = min(4 * c + 4, NKT)     # causal: k-tiles 0..4c+3
                nfull = 4 * c                 # k-tiles fully below the diagonal
                attn_n = {}

                def emit_scores(t, par):
                    # S^T and P^T for head (t,par): full k-tile pairs, then
                    # the 4 diagonal k-tiles with causal column trimming
                    av_full = []
                    av_diag = []
                    for g in range(nfull // 2):
                        st = stp.tile([P, 2 * QC], F32, tag="st")
                        for half in range(2):
                            j = 2 * g + half
                            nc.tensor.matmul(
                                st[:, half * QC:(half + 1) * QC],
                                kT_sb[t][par * DH:(par + 1) * DH,
                                         j * KT:(j + 1) * KT],
                                qT_t[(t, c)][par * DH:(par + 1) * DH, :],
                                start=True, stop=True)
                        pt = ptp.tile([P, 2 * QC], BF16, tag="pt")
                        nc.scalar.activation(
                            out=pt, in_=st,
                            func=mybir.ActivationFunctionType.Exp,
                            scale=float(scale))
                        av_full.append((2 * g, pt[:, 0:QC]))
                        av_full.append((2 * g + 1, pt[:, QC:2 * QC]))
                    for g in range(2):
                        st = stp.tile([P, 2 * QC], F32, tag="st")
                        for half in range(2):
                            m = 2 * g + half
                            j = nfull + m
                            lo = P * m
                            nc.tensor.matmul(
                                st[:, half * QC + lo:half * QC + QC],
                                kT_sb[t][par * DH:(par + 1) * DH,
                                         j * KT:(j + 1) * KT],
                                qT_t[(t, c)][par * DH:(par + 1) * DH, lo:QC],
                                start=True, stop=True)
                            pd = ptd[m][par]
                            nc.scalar.activation(
                                out=pd[:, lo:QC],
                                in_=st[:, half * QC + lo:half * QC + QC],
                                func=mybir.ActivationFunctionType.Exp,
                                scale=float(scale))
                            nc.gpsimd.tensor_mul(
                                pd[:, lo:lo + P], pd[:, lo:lo + P], tri_sb)
                            av_diag.append((j, pd, lo))
                    return (t, par, av_full, av_diag)

                def emit_av_norm(ctx):
                    t, par, av_full, av_diag = ctx
                    h = 2 * t + par
                    an_t = attn_n[t]
                    # AV with fused ones column -> rows 0..63 attn, row
                    # 64 = Z.  Diagonal tiles stream first (only their
                    # valid column suffix when a full tile follows to
                    # close the accumulation group); full tiles last.
                    acc = accp.tile([P, QC], F32, tag="acc")
                    for idx, (j, pd, lo) in enumerate(av_diag):
                        if nfull == 0:
                            lo = 0      # no full tile after: keep the
                                        # group full-width (prefix is 0)
                        nc.tensor.matmul(
                            acc[0:65, lo:QC],
                            v_sb[j][:, h, :],
                            pd[:, lo:QC],
                            start=(idx == 0),
                            stop=(nfull == 0 and idx == 3))
                    for idx, (j, rhs) in enumerate(av_full):
                        nc.tensor.matmul(
                            acc[0:65, :],
                            v_sb[j][:, h, :],
                            rhs,
                            start=False, stop=(idx == nfull - 1))
                    # normalization: broadcast Z over 64 rows via a DMA
                    # round-trip through DRAM (lane shift), then a fast
                    # approximate reciprocal on partitions 0..63
                    zraw = zrp.tile([P, QC], F32, tag="zraw")
                    bzs = bzsb.tile([P, QC], F32, tag="bzs")
                    zrow = zdp.tile([1, QC], F32, tag="zd", name="zrow")
                    # DMA cannot read PSUM: bounce Z through SBUF
                    nc.vector.tensor_copy(zraw[64:65, :], acc[64:65, :])
                    nc.sync.dma_start(out=zrow, in_=zraw[64:65, :])
                    nc.sync.dma_start(out=zraw[0:DH, :].unsqueeze(1),
                                      in_=zrow.partition_broadcast(DH))
                    nc.vector.reciprocal_approx_fast(
                        out=bzs[0:DH, :], in_=zraw[0:DH, :])
                    if par == 0:
                        nc.vector.tensor_mul(
                            an_t[0:DH, :], acc[0:DH, :], bzs[0:DH, :])
                    else:
                        an_o = anodd.tile([DH, QC], BF16, tag="anodd")
                        nc.vector.tensor_mul(
                            an_o, acc[0:DH, :], bzs[0:DH, :])
                        # shift to partitions 64..127 (DMA can cross lanes)
                        nc.sync.dma_start(out=an_t[DH:P, :], in_=an_o)

                def emit_oproj(oc):
                    # output projection for chunk oc + pipelined bf16
                    # pair-ReduceScatter + output slice write.  The last
                    # chunk runs in two column halves so the second (tail)
                    # ReduceScatter is half-sized.
                    an_c = attn_by_chunk.pop(oc)
                    if oc < NQC - 1:
                        pieces = [(partT[oc], rs_out[oc], 0, QC)]
                    else:
                        pieces = [(partTh[0], rs_outh[0], 0, QC // 2),
                                  (partTh[1], rs_outh[1], QC // 2, QC)]
                    last = oc == NQC - 1
                    for pT, rso, q0, q1 in pieces:
                        qn = q1 - q0
                        ot = otp.tile([P, D // P, QC], BF16, tag="ot")
                        for o in range(D // P):
                            po = scratch.tile([P, QC], F32, tag="pacc")
                            for t in range(HC // P):
                                nc.tensor.matmul(
                                    po[:, 0:qn],
                                    wo_sb[t][:, o * P:(o + 1) * P],
                                    an_c[t][:, q0:q1],
                                    start=(t == 0), stop=(t == HC // P - 1))
                            nc.vector.tensor_scalar_add(
                                ot[:, o, 0:qn], po[:, 0:qn], bo_sb[:, o:o + 1])
                            if last:
                                # per-o DMA: transfers pipeline with the
                                # remaining o-tiles, shortening the drain
                                # before the final ReduceScatter
                                nc.sync.dma_start(
                                    out=pT[o * P:(o + 1) * P, :],
                                    in_=ot[:, o, 0:qn])
                        if not last:
                            # one batched DMA for all 8 o-tiles
                            nc.sync.dma_start(
                                out=pT[:, :].rearrange("(o p) q -> p o q", o=D // P),
                                in_=ot[:, :, 0:qn])
                        nc.gpsimd.collective_compute(
                            "ReduceScatter", mybir.AluOpType.add,
                            replica_groups=[[0, 1], [2, 3], [4, 5], [6, 7]],
                            ins=[pT.opt()], outs=[rso.opt()],
                        )
                        nc.sync.dma_start(
                            out=outTh[oc * (D // 2):(oc + 1) * (D // 2), q0:q1],
                            in_=rso[:, :])

                # software-pipeline the heads: emit S^T/exp of head i+1
                # before the AV of head i so the in-order PE queue always
                # has independent matmuls while the Scalar engine runs exp
                pending = None
                for t in range(HC // P):
                    attn_n[t] = anp.tile([P, QC], BF16, tag="an", name="an_t")
                    # par=1 first: its lane-shift DMA then overlaps par=0's
                    # compute, and the an tile's last writer is par=0's
                    # direct DVE mul (no DMA on the O-projection's
                    # critical path)
                    for par in (1, 0):
                        ctx = emit_scores(t, par)
                        if s == 0 and t == 0 and par == 1:
                            # deferred wo fetch: these DMA issues sit behind
                            # the first head's gpsimd tri-muls, keeping the
                            # startup rings free for wq/x/wk/wv
                            for wt in range(HC // P):
                                nc.gpsimd.dma_start(
                                    out=wo_sb[wt], in_=wo[wt * P:(wt + 1) * P, :])
                        if pending is not None:
                            emit_av_norm(pending)
                        pending = ctx
                attn_by_chunk[c] = attn_n
                # previous chunk's O-projection fills the PE while the last
                # head's exp runs; then flush the pending AV
                if c - 1 >= 0:
                    emit_oproj(c - 1)
                emit_av_norm(pending)
                if c == NQC - 1:
                    emit_oproj(c)

    nc.compile()
    return nc


def _make_in_maps(x, Wq, bq, Wk, bk, Wv, bv, Wo, bo, mask):
    ref = np.tril(np.ones((L, L), dtype=np.int32))[None, None]
    assert np.array_equal(np.asarray(mask), ref), "mask must be causal"

    # triangle pattern for the diagonal [128,128] block: key p attends q f
    # iff p <= f
    tri = (np.arange(P)[:, None] <= np.arange(P)[None, :]).astype(np.float32)

    in_maps = []
    for c in range(N_CORES):
        b, g = c // 2, c % 2
        cols = slice(HC * g, HC * g + HC)
        # chunk-major x^T: [NQC*D, QC], row s*D + d = x^T[d, s*QC:(s+1)*QC]
        xTb = np.asarray(x[b]).T.reshape(D, L // QC, QC).transpose(1, 0, 2)
        in_maps.append({
            "xT": np.ascontiguousarray(xTb.reshape(-1, QC)).astype(ml_dtypes.bfloat16),
            "wq": np.ascontiguousarray(np.asarray(Wq)[:, cols]).astype(ml_dtypes.bfloat16),
            "wk": np.ascontiguousarray(np.asarray(Wk)[:, cols]).astype(ml_dtypes.bfloat16),
            "wv": np.ascontiguousarray(np.asarray(Wv)[:, cols]).astype(ml_dtypes.bfloat16),
            "wo": np.ascontiguousarray(np.asarray(Wo)[cols, :]).astype(ml_dtypes.bfloat16),
            "bq": np.ascontiguousarray(np.asarray(bq)[cols].reshape(HC // P, P).T),
            "bk": np.ascontiguousarray(np.asarray(bk)[cols].reshape(HC // P, P).T),
            "bv": np.ascontiguousarray(np.asarray(bv)[cols].reshape(1, HC)),
            "bo": np.ascontiguousarray(
                (np.asarray(bo) / 2.0).reshape(D // P, P).T.astype(np.float32)),
            "tri": tri.astype(ml_dtypes.bfloat16),
        })
    return in_maps


def kernel(x, Wq, bq, Wk, bk, Wv, bv, Wo, bo, mask):
    global _NC, LAST_EXEC_NS
    if _NC is None:
        _NC = build_nc()
    in_maps = _make_in_maps(x, Wq, bq, Wk, bk, Wv, bv, Wo, bo, mask)
    r = run_bass_kernel_spmd(
        _NC, in_maps, core_ids=list(range(N_CORES)), trace=TRACE)
    LAST_EXEC_NS = r.exec_time_ns
    out = np.empty((B, L, D), dtype=np.float32)
    for b in range(B):
        # outTh is chunk-major [NQC*(D//2), QC] -> [D//2, L]
        halves = []
        for cc in (2 * b, 2 * b + 1):
            oT = r.results[cc]["outTh"].astype(np.float32)
            halves.append(
                oT.reshape(L // QC, D // 2, QC).transpose(1, 0, 2).reshape(D // 2, L))
        out[b] = np.concatenate(halves, axis=0).T
    return out
